# revision 8
# baseline (speedup 1.0000x reference)
"""Trainium2 Bass kernel for nn_MoETransformerEncoderLayer_52750788329547.

Sharding: token-parallel across 8 NeuronCores. Each core owns 256 tokens
(batch c//4, row block c%4), runs LN1 + GQA attention against its full batch
(keys/values recomputed locally), LN2, noisy-top-2 gating, and the dense
8-expert MoE combine for its tokens. No device collectives. The two scalar
aux-loss reductions (E=8 column sums) are finished on the host from per-core
(256,8) outputs.

Precision: everything that feeds the top-k routing (attention -> LN2 ->
gating logits / noise scale) is computed in fp32 (min top-2/3 gap in Hn is
~4e-5, so bf16 there would flip expert routing vs the reference). The expert
matmuls / xV / W2 run in bf16 with fp32 PSUM accumulation.

Per-core layouts place tokens on SBUF partitions for LN/softmax-denominator/
gating reductions, and features on partitions for matmul stationary operands;
PE transposes (via identity) bridge the two.
"""
import os
import sys

sys.path.insert(0, "/opt/trn_rl_repo")

import numpy as np
import ml_dtypes

import concourse.bass as bass
import concourse.mybir as mybir
from concourse import bacc, tile
from concourse.bass_utils import run_bass_kernel_spmd
from concourse.masks import make_identity

dt = mybir.dt
AF = mybir.ActivationFunctionType
OP = mybir.AluOpType
AX = mybir.AxisListType

P = 128
M = 1024          # model dim
DH = 4096         # expert hidden dim
E = 8             # experts
NH = 16           # heads
NG = 4            # kv groups
HD = 64           # head dim
KV = NG * HD      # 256
NB = 1024         # tokens per batch
TL = 256          # tokens per core
SCALE = HD ** -0.5
BIG = 1e30
EPS = 1e-5

MT = M // P       # 8 m-tiles
TT = TL // P      # 2 local token tiles
NT = NB // P      # 8 batch token tiles
DC = DH // 512    # 8 dh chunks of 512
DT = DH // P      # 32 dh tiles of 128

_programs = {}

# float32r measured at rel~1.6e-4 on HW — too coarse for the routing chain
# (min top-k gap ~4e-5), so plain fp32 matmuls are the default there.
USE_F32R = os.environ.get("KERNEL_F32R", "0") == "1"


def _ln_stats(nc, pool, xt_ap, width):
    """Return (rsig, nmr) [P,1] f32 tiles: y = x*rsig + nmr is LayerNorm(x)
    (gain/bias folded into downstream weights). Newton-refined rsqrt."""
    ssum = pool.tile([P, 1], dt.float32, tag="ssum", name="ssum")
    nc.vector.reduce_sum(ssum[:], xt_ap, axis=AX.X)
    sq = pool.tile([P, width], dt.float32, tag="sqscratch", name="sq")
    ssq = pool.tile([P, 1], dt.float32, tag="ssq", name="ssq")
    nc.scalar.activation(sq[:], xt_ap, AF.Square, accum_out=ssq[:])
    mu = pool.tile([P, 1], dt.float32, tag="mu", name="mu")
    nc.vector.tensor_scalar_mul(mu[:], ssum[:], 1.0 / width)
    ex2 = pool.tile([P, 1], dt.float32, tag="ex2", name="ex2")
    nc.vector.tensor_scalar_mul(ex2[:], ssq[:], 1.0 / width)
    mu2 = pool.tile([P, 1], dt.float32, tag="mu2", name="mu2")
    nc.vector.tensor_mul(mu2[:], mu[:], mu[:])
    ve = pool.tile([P, 1], dt.float32, tag="ve", name="ve")
    nc.vector.tensor_sub(ve[:], ex2[:], mu2[:])
    nc.vector.tensor_scalar_add(ve[:], ve[:], EPS)
    s0 = pool.tile([P, 1], dt.float32, tag="s0", name="s0")
    nc.scalar.activation(s0[:], ve[:], AF.Sqrt)
    r0 = pool.tile([P, 1], dt.float32, tag="r0", name="r0")
    nc.vector.reciprocal(r0[:], s0[:])
    vr = pool.tile([P, 1], dt.float32, tag="vr", name="vr")
    nc.vector.tensor_mul(vr[:], ve[:], r0[:])
    s1h = pool.tile([P, 1], dt.float32, tag="s1h", name="s1h")
    nc.vector.tensor_add(s1h[:], s0[:], vr[:])  # = 2*sqrt(ve) after Newton
    rsig = pool.tile([P, 1], dt.float32, tag="rsig", name="rsig")
    nc.vector.reciprocal(rsig[:], s1h[:])
    nc.vector.tensor_scalar_mul(rsig[:], rsig[:], 2.0)
    nmr = pool.tile([P, 1], dt.float32, tag="nmr", name="nmr")
    nc.vector.tensor_scalar(nmr[:], mu[:], rsig[:], -1.0, OP.mult, OP.mult)
    return rsig, nmr


def _build_program(flags):
    has_bq, has_bk, has_bv, has_bg, has_bn, has_be, has_vb, has_w2b = flags
    f32 = dt.float32
    bf16 = dt.bfloat16
    fr = dt.float32r if USE_F32R else dt.float32

    nc = bacc.Bacc("TRN2", target_bir_lowering=False, debug=False,
                   num_devices=8)

    # ---- I/O ----
    xb = nc.dram_tensor("xb", [NB, M], f32, kind="ExternalInput")
    xpb = nc.dram_tensor("xpb", [TL, M], f32, kind="ExternalInput")
    noise_d = nc.dram_tensor("noise", [TL, E], f32, kind="ExternalInput")
    wq_d = nc.dram_tensor("wq", [M, M], f32, kind="ExternalInput")
    wk_d = nc.dram_tensor("wk", [M, KV], f32, kind="ExternalInput")
    wv_d = nc.dram_tensor("wv", [M, KV], f32, kind="ExternalInput")
    wo_d = nc.dram_tensor("wo", [M, M], f32, kind="ExternalInput")
    wg_d = nc.dram_tensor("wg", [M, E], f32, kind="ExternalInput")
    wn_d = nc.dram_tensor("wn", [M, E], f32, kind="ExternalInput")
    we_d = nc.dram_tensor("we", [E, M, DH], bf16, kind="ExternalInput")
    vw_d = nc.dram_tensor("vw", [M, DH], bf16, kind="ExternalInput")
    w2_d = nc.dram_tensor("w2", [DH, M], bf16, kind="ExternalInput")
    bqT_d = bkT_d = bvr_d = bgr_d = bnr_d = ber_d = vbr_d = w2br_d = None
    if has_bq:
        bqT_d = nc.dram_tensor("bqT", [P, MT], f32, kind="ExternalInput")
    if has_bk:
        bkT_d = nc.dram_tensor("bkT", [P, KV // P], f32, kind="ExternalInput")
    if has_bv:
        bvr_d = nc.dram_tensor("bvr", [P, KV], f32, kind="ExternalInput")
    if has_bg:
        bgr_d = nc.dram_tensor("bgr", [P, E], f32, kind="ExternalInput")
    if has_bn:
        bnr_d = nc.dram_tensor("bnr", [P, E], f32, kind="ExternalInput")
    if has_be:
        ber_d = nc.dram_tensor("ber", [E, DH], f32, kind="ExternalInput")
    if has_vb:
        vbr_d = nc.dram_tensor("vbr", [P, DH], f32, kind="ExternalInput")
    if has_w2b:
        w2br_d = nc.dram_tensor("w2br", [P, M], f32, kind="ExternalInput")

    out_d = nc.dram_tensor("out", [TL, M], f32, kind="ExternalOutput")
    u_d = nc.dram_tensor("u", [TL, E], f32, kind="ExternalOutput")
    gates_d = nc.dram_tensor("gates", [TL, E], f32, kind="ExternalOutput")

    from contextlib import ExitStack

    with tile.TileContext(nc) as tc, ExitStack() as top:
        const = top.enter_context(tc.tile_pool(name="const", bufs=1))
        id32 = const.tile([P, P], f32, name="id32")
        make_identity(nc, id32[:])
        idb = const.tile([P, P], bf16, name="idb")
        make_identity(nc, idb[:])

        # persistent activation buffers
        persist = top.enter_context(tc.tile_pool(name="persist", bufs=1))
        x2_all = persist.tile([P, TT * M], f32, name="x2_all")
        xpb_all = persist.tile([P, TT * M], f32, name="xpb_all")
        for t in range(TT):
            nc.sync.dma_start(xpb_all[:, t * M:(t + 1) * M],
                              xpb[t * P:(t + 1) * P, :])

        # ============ Phase 1: LN1 over the full batch ============
        with ExitStack() as ph:
            ypool = ph.enter_context(tc.tile_pool(name="ypool", bufs=1))
            y_all = ypool.tile([P, NT * M], f32, name="y_all")
            with ExitStack() as inner:
                xin = inner.enter_context(tc.tile_pool(name="xin", bufs=3))
                lns = inner.enter_context(tc.tile_pool(name="lns", bufs=3))
                for t in range(NT):
                    xt = xin.tile([P, M], f32, tag="xt", name="xt")
                    nc.sync.dma_start(xt[:], xb[t * P:(t + 1) * P, :])
                    rsig, nmr = _ln_stats(nc, lns, xt[:], M)
                    nc.scalar.activation(y_all[:, t * M:(t + 1) * M], xt[:],
                                         AF.Identity, bias=nmr[:],
                                         scale=rsig[:])

            # ============ Phase 2: y^T ============
            ytp = ph.enter_context(tc.tile_pool(name="ytp", bufs=1))
            yT = ytp.tile([P, NT * M], fr, name="yT")
            with ExitStack() as inner:
                tps = inner.enter_context(
                    tc.tile_pool(name="tps", bufs=4, space="PSUM"))
                for t in range(NT):
                    for mt in range(MT):
                        pst = tps.tile([P, P], f32, tag="pst", name="pst")
                        nc.tensor.transpose(
                            pst[:], y_all[:, t * M + mt * P: t * M + mt * P + P],
                            id32[:])
                        nc.scalar.copy(
                            yT[:, mt * NB + t * P: mt * NB + t * P + P], pst[:])

            # ============ Phase 3: q^T, k^T, v ============
            attn = ph.enter_context(tc.tile_pool(name="attn", bufs=1))
            qT = attn.tile([P, MT * TL], fr, name="qT")
            kT = attn.tile([P, (KV // P) * NB], fr, name="kT")
            v_all = attn.tile([P, NT * (NG * (HD + 1))], fr, name="v_all")
            GW = NG * (HD + 1)  # 260 columns per key tile

            with ExitStack() as inner:
                wst = inner.enter_context(tc.tile_pool(name="wst", bufs=4))
                qps = inner.enter_context(
                    tc.tile_pool(name="qps", bufs=2, space="PSUM"))
                bq_sb = None
                if has_bq:
                    bq_sb = attn.tile([P, MT], f32, name="bq_sb")
                    nc.sync.dma_start(bq_sb[:], bqT_d[:, :])
                bk_sb = None
                if has_bk:
                    bk_sb = attn.tile([P, KV // P], f32, name="bk_sb")
                    nc.sync.dma_start(bk_sb[:], bkT_d[:, :])
                bv_sb = None
                if has_bv:
                    bv_sb = attn.tile([P, KV], f32, name="bv_sb")
                    nc.sync.dma_start(bv_sb[:], bvr_d[:, :])

                def wtile(dram_ap, tag):
                    """DMA a weight tile; convert to f32r via DVE if needed."""
                    t0 = wst.tile([P, dram_ap.shape[-1]], f32, tag=tag,
                                  name=tag)
                    nc.sync.dma_start(t0[:], dram_ap)
                    if not USE_F32R:
                        return t0
                    t1 = wst.tile([P, dram_ap.shape[-1]], fr, tag=tag + "r",
                                  name=tag + "r")
                    nc.vector.tensor_copy(t1[:], t0[:])
                    return t1

                # q^T (features on partitions, local 256 tokens on free)
                for ht in range(MT):
                    psq = qps.tile([P, TL], f32, tag="psq", name="psq")
                    for mt in range(MT):
                        wt = wtile(wq_d[mt * P:(mt + 1) * P,
                                        ht * P:(ht + 1) * P], "wq")
                        nc.tensor.matmul(psq[:], wt[:],
                                         yT[:, mt * NB: mt * NB + TL],
                                         start=(mt == 0), stop=(mt == MT - 1))
                    if has_bq:
                        nc.scalar.activation(qT[:, ht * TL:(ht + 1) * TL],
                                             psq[:], AF.Identity,
                                             bias=bq_sb[:, ht:ht + 1])
                    else:
                        nc.scalar.copy(qT[:, ht * TL:(ht + 1) * TL], psq[:])

                # k^T (kv features on partitions, all 1024 batch tokens free)
                for kt in range(KV // P):
                    psk = [qps.tile([P, 512], f32, tag=f"psk{ch}",
                                    name=f"psk{ch}") for ch in range(2)]
                    for mt in range(MT):
                        wt = wtile(wk_d[mt * P:(mt + 1) * P,
                                        kt * P:(kt + 1) * P], "wk")
                        for ch in range(2):
                            nc.tensor.matmul(
                                psk[ch][:], wt[:],
                                yT[:, mt * NB + ch * 512: mt * NB + ch * 512 + 512],
                                start=(mt == 0), stop=(mt == MT - 1))
                    for ch in range(2):
                        dst = kT[:, kt * NB + ch * 512: kt * NB + ch * 512 + 512]
                        if has_bk:
                            nc.scalar.activation(dst, psk[ch][:], AF.Identity,
                                                 bias=bk_sb[:, kt:kt + 1])
                        else:
                            nc.scalar.copy(dst, psk[ch][:])

                # v natural (batch tokens on partitions) with ones column per group
                wv_sb = attn.tile([P, MT * KV], f32, name="wv_sb")
                for mt in range(MT):
                    nc.sync.dma_start(wv_sb[:, mt * KV:(mt + 1) * KV],
                                      wv_d[mt * P:(mt + 1) * P, :])
                for t in range(NT):
                    psv = qps.tile([P, KV], f32, tag="psv", name="psv")
                    for mt in range(MT):
                        nc.tensor.matmul(
                            psv[:],
                            yT[:, mt * NB + t * P: mt * NB + t * P + P],
                            wv_sb[:, mt * KV:(mt + 1) * KV],
                            start=(mt == 0), stop=(mt == MT - 1))
                    base = t * GW
                    nc.vector.memset(v_all[:, base: base + GW], 1.0)
                    for g in range(NG):
                        dst = v_all[:, base + g * (HD + 1): base + g * (HD + 1) + HD]
                        if has_bv:
                            nc.vector.scalar_tensor_tensor(
                                dst, psv[:, g * HD:(g + 1) * HD], 1.0,
                                bv_sb[:, g * HD:(g + 1) * HD], OP.mult, OP.add)
                        else:
                            nc.vector.tensor_copy(dst, psv[:, g * HD:(g + 1) * HD])

            # ============ Phase 4: attention heads ============
            aT = attn.tile([P, MT * TL], fr, name="aT")
            with ExitStack() as inner:
                ptp = inner.enter_context(tc.tile_pool(name="ptp", bufs=2))
                sps = inner.enter_context(
                    tc.tile_pool(name="sps", bufs=3, space="PSUM"))
                aps = inner.enter_context(
                    tc.tile_pool(name="aps", bufs=2, space="PSUM"))
                hsm = inner.enter_context(tc.tile_pool(name="hsm", bufs=2))
                for h in range(NH):
                    g = h % NG
                    krow = (g % 2) * HD
                    kcol = (g // 2) * NB
                    qrow = (h % 2) * HD
                    qcol = (h // 2) * TL
                    PTt = ptp.tile([P, NT * TL], fr, tag="PT", name="PTt")
                    for k8 in range(NT):
                        pss = sps.tile([P, TL], f32, tag="pss", name="pss")
                        nc.tensor.matmul(
                            pss[:],
                            kT[krow:krow + HD, kcol + k8 * P: kcol + k8 * P + P],
                            qT[qrow:qrow + HD, qcol: qcol + TL],
                            start=True, stop=True)
                        nc.scalar.activation(PTt[:, k8 * TL:(k8 + 1) * TL],
                                             pss[:], AF.Exp, scale=SCALE)
                    psa = aps.tile([HD + 1, TL], f32, tag="psa", name="psa")
                    for k8 in range(NT):
                        nc.tensor.matmul(
                            psa[:],
                            v_all[:, k8 * GW + g * (HD + 1): k8 * GW + (g + 1) * (HD + 1)],
                            PTt[:, k8 * TL:(k8 + 1) * TL],
                            start=(k8 == 0), stop=(k8 == NT - 1))
                    rr = hsm.tile([1, TL], f32, tag="rr", name="rr")
                    nc.vector.reciprocal(rr[:], psa[HD:HD + 1, :])
                    rb = hsm.tile([HD, TL], f32, tag="rb", name="rb")
                    nc.gpsimd.partition_broadcast(rb[:], rr[:])
                    nc.vector.tensor_mul(
                        aT[qrow:qrow + HD, qcol: qcol + TL],
                        psa[0:HD, :], rb[:])

            # ============ Phase 5: Wo + residual ============
            with ExitStack() as inner:
                wst = inner.enter_context(tc.tile_pool(name="wst2", bufs=4))
                wps = inner.enter_context(
                    tc.tile_pool(name="wps", bufs=4, space="PSUM"))
                for mc in range(2):
                    psw = [wps.tile([P, 512], f32, tag=f"psw{t}",
                                    name=f"psw{t}") for t in range(TT)]
                    for kt in range(MT):
                        t0 = wst.tile([P, 512], f32, tag="wo", name="wo_t")
                        nc.sync.dma_start(
                            t0[:], wo_d[kt * P:(kt + 1) * P,
                                        mc * 512:(mc + 1) * 512])
                        if USE_F32R:
                            wt = wst.tile([P, 512], fr, tag="wor", name="wor_t")
                            nc.vector.tensor_copy(wt[:], t0[:])
                        else:
                            wt = t0
                        for t in range(TT):
                            nc.tensor.matmul(
                                psw[t][:],
                                aT[:, kt * TL + t * P: kt * TL + t * P + P],
                                wt[:], start=(kt == 0), stop=(kt == MT - 1))
                    for t in range(TT):
                        nc.vector.tensor_add(
                            x2_all[:, t * M + mc * 512: t * M + mc * 512 + 512],
                            psw[t][:],
                            xpb_all[:, t * M + mc * 512: t * M + mc * 512 + 512])

        # ============ Phase 6: LN2 -> z, z^T ============
        moe = top.enter_context(tc.tile_pool(name="moe", bufs=1))
        zT32 = moe.tile([P, MT * TL], f32, name="zT32")
        zTb = moe.tile([P, MT * TL], bf16, name="zTb")
        with ExitStack() as inner:
            lns = inner.enter_context(tc.tile_pool(name="lns2", bufs=2))
            zp = inner.enter_context(tc.tile_pool(name="zp", bufs=1))
            z_all = zp.tile([P, TT * M], f32, name="z_all")
            for t in range(TT):
                rsig, nmr = _ln_stats(nc, lns, x2_all[:, t * M:(t + 1) * M], M)
                nc.scalar.activation(z_all[:, t * M:(t + 1) * M],
                                     x2_all[:, t * M:(t + 1) * M],
                                     AF.Identity, bias=nmr[:], scale=rsig[:])
            tps = inner.enter_context(
                tc.tile_pool(name="tps2", bufs=4, space="PSUM"))
            for t in range(TT):
                for mt in range(MT):
                    pst = tps.tile([P, P], f32, tag="pst", name="pst2")
                    nc.tensor.transpose(
                        pst[:], z_all[:, t * M + mt * P: t * M + mt * P + P],
                        id32[:])
                    nc.scalar.copy(
                        zT32[:, mt * TL + t * P: mt * TL + t * P + P], pst[:])
                    nc.vector.tensor_copy(
                        zTb[:, mt * TL + t * P: mt * TL + t * P + P], pst[:])

        # ============ Phase 7: gating ============
        gates_all = moe.tile([P, TT * E], f32, name="gates_all")
        with ExitStack() as inner:
            gsb = inner.enter_context(tc.tile_pool(name="gsb", bufs=2))
            gps = inner.enter_context(
                tc.tile_pool(name="gps", bufs=2, space="PSUM"))
            wg_sb = gsb.tile([P, MT * E], f32, tag="wg", name="wg_sb")
            wn_sb = gsb.tile([P, MT * E], f32, tag="wn", name="wn_sb")
            for mt in range(MT):
                nc.sync.dma_start(wg_sb[:, mt * E:(mt + 1) * E],
                                  wg_d[mt * P:(mt + 1) * P, :])
                nc.sync.dma_start(wn_sb[:, mt * E:(mt + 1) * E],
                                  wn_d[mt * P:(mt + 1) * P, :])
            bg_sb = bn_sb = None
            if has_bg:
                bg_sb = gsb.tile([P, E], f32, tag="bg", name="bg_sb")
                nc.sync.dma_start(bg_sb[:], bgr_d[:, :])
            if has_bn:
                bn_sb = gsb.tile([P, E], f32, tag="bn", name="bn_sb")
                nc.sync.dma_start(bn_sb[:], bnr_d[:, :])

            for t in range(TT):
                psl = gps.tile([P, E], f32, tag="psl", name="psl")
                psn = gps.tile([P, E], f32, tag="psn", name="psn")
                for mt in range(MT):
                    lhsT = zT32[:, mt * TL + t * P: mt * TL + t * P + P]
                    nc.tensor.matmul(psl[:], lhsT, wg_sb[:, mt * E:(mt + 1) * E],
                                     start=(mt == 0), stop=(mt == MT - 1))
                    nc.tensor.matmul(psn[:], lhsT, wn_sb[:, mt * E:(mt + 1) * E],
                                     start=(mt == 0), stop=(mt == MT - 1))

                def gt(tag):
                    return gsb.tile([P, E], f32, tag=tag, name=tag)

                logits = gt("logits")
                if has_bg:
                    nc.vector.tensor_add(logits[:], psl[:], bg_sb[:])
                else:
                    nc.scalar.copy(logits[:], psl[:])
                raw = gt("raw")
                if has_bn:
                    nc.vector.tensor_add(raw[:], psn[:], bn_sb[:])
                else:
                    nc.scalar.copy(raw[:], psn[:])
                # ns = softplus(raw) = relu(raw) + ln(1 + exp(-|raw|))
                t_abs = gt("t_abs")
                nc.scalar.activation(t_abs[:], raw[:], AF.Abs)
                t_exp = gt("t_exp")
                nc.scalar.activation(t_exp[:], t_abs[:], AF.Exp, scale=-1.0)
                t_ln = gt("t_ln")
                nc.scalar.activation(t_ln[:], t_exp[:], AF.Ln, bias=1.0)
                t_rel = gt("t_rel")
                nc.scalar.activation(t_rel[:], raw[:], AF.Relu)
                ns = gt("ns")
                nc.vector.tensor_add(ns[:], t_rel[:], t_ln[:])
                noise_sb = gt("noise_sb")
                nc.sync.dma_start(noise_sb[:], noise_d[t * P:(t + 1) * P, :])
                Hn = gt("Hn")
                nc.vector.tensor_mul(Hn[:], noise_sb[:], ns[:])
                nc.vector.tensor_add(Hn[:], Hn[:], logits[:])

                def col(tag):
                    return gsb.tile([P, 1], f32, tag=tag, name=tag)

                m1 = col("m1")
                nc.vector.reduce_max(m1[:], Hn[:], axis=AX.X)
                mask1 = gt("mask1")
                nc.vector.tensor_scalar(mask1[:], Hn[:], m1[:], None, OP.is_ge)
                Hn2 = gt("Hn2")
                nc.vector.scalar_tensor_tensor(Hn2[:], mask1[:], -BIG, Hn[:],
                                               OP.mult, OP.add)
                m2 = col("m2")
                nc.vector.reduce_max(m2[:], Hn2[:], axis=AX.X)
                mask2 = gt("mask2")
                nc.vector.tensor_scalar(mask2[:], Hn2[:], m2[:], None, OP.is_ge)
                Hn3 = gt("Hn3")
                nc.vector.scalar_tensor_tensor(Hn3[:], mask2[:], -BIG, Hn2[:],
                                               OP.mult, OP.add)
                m3 = col("m3")
                nc.vector.reduce_max(m3[:], Hn3[:], axis=AX.X)
                # gates = mask1*sig(m1-m2) + mask2*(1-sig(m1-m2))
                d = col("d")
                nc.vector.tensor_sub(d[:], m2[:], m1[:])
                ed = col("ed")
                nc.scalar.activation(ed[:], d[:], AF.Exp)
                nc.vector.tensor_scalar_add(ed[:], ed[:], 1.0)
                g1 = col("g1")
                nc.vector.reciprocal(g1[:], ed[:])
                g2 = col("g2")
                nc.scalar.activation(g2[:], g1[:], AF.Copy, bias=1.0,
                                     scale=-1.0)
                t1g = gt("t1g")
                nc.vector.tensor_scalar(t1g[:], mask1[:], g1[:], None, OP.mult)
                nc.vector.scalar_tensor_tensor(
                    gates_all[:, t * E:(t + 1) * E], mask2[:], g2[:], t1g[:],
                    OP.mult, OP.add)
                # psi = m3 + (mask1+mask2)*(m2-m3); u = (logits-psi)/ns
                msum = gt("msum")
                nc.vector.tensor_add(msum[:], mask1[:], mask2[:])
                d23 = col("d23")
                nc.vector.tensor_sub(d23[:], m2[:], m3[:])
                psi = gt("psi")
                nc.vector.tensor_scalar(psi[:], msum[:], d23[:], m3[:],
                                        OP.mult, OP.add)
                lp = gt("lp")
                nc.vector.tensor_sub(lp[:], logits[:], psi[:])
                rns = gt("rns")
                nc.vector.reciprocal(rns[:], ns[:])
                uu = gt("uu")
                nc.vector.tensor_mul(uu[:], lp[:], rns[:])
                nc.sync.dma_start(u_d[t * P:(t + 1) * P, :], uu[:])
                nc.sync.dma_start(gates_d[t * P:(t + 1) * P, :],
                                  gates_all[:, t * E:(t + 1) * E])

        # ============ Phase 8: xV ============
        s_all = moe.tile([P, TT * DH], f32, name="s_all")
        xv_all = moe.tile([P, TT * DH], bf16, name="xv_all")
        nc.gpsimd.memset(s_all[:], 0.0)
        with ExitStack() as inner:
            wst = inner.enter_context(tc.tile_pool(name="wst3", bufs=4))
            xps = inner.enter_context(
                tc.tile_pool(name="xps", bufs=4, space="PSUM"))
            vb_sb = None
            if has_vb:
                vbp = inner.enter_context(tc.tile_pool(name="vbp", bufs=1))
                vb_sb = vbp.tile([P, DH], f32, name="vb_sb")
                nc.sync.dma_start(vb_sb[:], vbr_d[:, :])
            for dc in range(DC):
                psx = [xps.tile([P, 512], f32, tag=f"psx{t}",
                                name=f"psx{t}") for t in range(TT)]
                for mt in range(MT):
                    vt = wst.tile([P, 512], bf16, tag="vwt", name="vwt")
                    nc.sync.dma_start(vt[:], vw_d[mt * P:(mt + 1) * P,
                                                  dc * 512:(dc + 1) * 512])
                    for t in range(TT):
                        nc.tensor.matmul(
                            psx[t][:],
                            zTb[:, mt * TL + t * P: mt * TL + t * P + P],
                            vt[:], start=(mt == 0), stop=(mt == MT - 1))
                for t in range(TT):
                    dst = xv_all[:, t * DH + dc * 512: t * DH + dc * 512 + 512]
                    if has_vb:
                        nc.vector.tensor_add(
                            dst, psx[t][:], vb_sb[:, dc * 512:(dc + 1) * 512])
                    else:
                        nc.scalar.copy(dst, psx[t][:])

        # ============ Phase 9: experts (dense, bf16) ============
        with ExitStack() as inner:
            wst = inner.enter_context(tc.tile_pool(name="wst4", bufs=6))
            eps_ = inner.enter_context(
                tc.tile_pool(name="eps", bufs=4, space="PSUM"))
            spool = inner.enter_context(tc.tile_pool(name="spool", bufs=3))
            be_sb = None
            if has_be:
                bep = inner.enter_context(tc.tile_pool(name="bep", bufs=2))
            for e in range(E):
                if has_be:
                    ber_row = bep.tile([1, DH], f32, tag="ber_row",
                                       name="ber_row")
                    nc.sync.dma_start(ber_row[:], ber_d[e:e + 1, :])
                    be_sb = bep.tile([P, DH], f32, tag="be_sb", name="be_sb")
                    nc.gpsimd.partition_broadcast(be_sb[:], ber_row[:])
                for dc in range(DC):
                    pse = [eps_.tile([P, 512], f32, tag=f"pse{t}",
                                     name=f"pse{t}") for t in range(TT)]
                    for mt in range(MT):
                        wt = wst.tile([P, 512], bf16, tag="wet", name="wet")
                        nc.sync.dma_start(
                            wt[:], we_d[e, mt * P:(mt + 1) * P,
                                        dc * 512:(dc + 1) * 512])
                        for t in range(TT):
                            nc.tensor.matmul(
                                pse[t][:],
                                zTb[:, mt * TL + t * P: mt * TL + t * P + P],
                                wt[:], start=(mt == 0), stop=(mt == MT - 1))
                    for t in range(TT):
                        if has_be:
                            nc.vector.tensor_add(
                                pse[t][:], pse[t][:],
                                be_sb[:, dc * 512:(dc + 1) * 512])
                        sil = spool.tile([P, 512], bf16, tag="sil", name="sil")
                        nc.scalar.activation(sil[:], pse[t][:], AF.Silu)
                        sl = s_all[:, t * DH + dc * 512: t * DH + dc * 512 + 512]
                        nc.vector.scalar_tensor_tensor(
                            sl, sil[:], gates_all[:, t * E + e: t * E + e + 1],
                            sl, OP.mult, OP.add)

        # ============ Phase 10: gated, gated^T ============
        gT = moe.tile([P, DT * TL], bf16, name="gT")
        with ExitStack() as inner:
            gp = inner.enter_context(tc.tile_pool(name="gp", bufs=2))
            tps = inner.enter_context(
                tc.tile_pool(name="tps3", bufs=4, space="PSUM"))
            for t in range(TT):
                gtmp = gp.tile([P, DH], bf16, tag="gtmp", name="gtmp")
                nc.vector.tensor_mul(gtmp[:], s_all[:, t * DH:(t + 1) * DH],
                                     xv_all[:, t * DH:(t + 1) * DH])
                for dtt in range(DT):
                    pst = tps.tile([P, P], bf16, tag="pst", name="pst3")
                    nc.tensor.transpose(pst[:], gtmp[:, dtt * P:(dtt + 1) * P],
                                        idb[:])
                    nc.scalar.copy(
                        gT[:, dtt * TL + t * P: dtt * TL + t * P + P], pst[:])

        # ============ Phase 11: W2 + residual out ============
        with ExitStack() as inner:
            wst = inner.enter_context(tc.tile_pool(name="wst5", bufs=6))
            fps = inner.enter_context(
                tc.tile_pool(name="fps", bufs=4, space="PSUM"))
            opool = inner.enter_context(tc.tile_pool(name="opool", bufs=3))
            w2b_sb = None
            if has_w2b:
                wbp = inner.enter_context(tc.tile_pool(name="wbp", bufs=1))
                w2b_sb = wbp.tile([P, M], f32, name="w2b_sb")
                nc.sync.dma_start(w2b_sb[:], w2br_d[:, :])
            for mc in range(2):
                psf = [fps.tile([P, 512], f32, tag=f"psf{t}",
                                name=f"psf{t}") for t in range(TT)]
                for dtt in range(DT):
                    w2t = wst.tile([P, 512], bf16, tag="w2t", name="w2t")
                    nc.sync.dma_start(w2t[:], w2_d[dtt * P:(dtt + 1) * P,
                                                   mc * 512:(mc + 1) * 512])
                    for t in range(TT):
                        nc.tensor.matmul(
                            psf[t][:],
                            gT[:, dtt * TL + t * P: dtt * TL + t * P + P],
                            w2t[:], start=(dtt == 0), stop=(dtt == DT - 1))
                for t in range(TT):
                    ot = opool.tile([P, 512], f32, tag="ot", name="ot")
                    nc.vector.tensor_add(
                        ot[:], psf[t][:],
                        x2_all[:, t * M + mc * 512: t * M + mc * 512 + 512])
                    if has_w2b:
                        nc.vector.tensor_add(
                            ot[:], ot[:], w2b_sb[:, mc * 512:(mc + 1) * 512])
                    nc.sync.dma_start(
                        out_d[t * P:(t + 1) * P, mc * 512:(mc + 1) * 512],
                        ot[:])

    nc.compile()
    return nc


def _get_program(flags):
    if flags not in _programs:
        _programs[flags] = _build_program(flags)
    return _programs[flags]


def kernel(**inputs):
    f32 = np.float32
    bf = ml_dtypes.bfloat16
    x = np.ascontiguousarray(np.asarray(inputs["x"], dtype=f32))
    noise = np.asarray(inputs["noise"], dtype=f32)
    ln1_g = np.asarray(inputs["ln1_g"], f32)
    ln1_b = np.asarray(inputs["ln1_b"], f32)
    ln2_g = np.asarray(inputs["ln2_g"], f32)
    ln2_b = np.asarray(inputs["ln2_b"], f32)
    Wq = np.asarray(inputs["Wq"], f32)
    bq = np.asarray(inputs["bq"], f32)
    Wk = np.asarray(inputs["Wk"], f32)
    bk = np.asarray(inputs["bk"], f32)
    Wv = np.asarray(inputs["Wv"], f32)
    bv = np.asarray(inputs["bv"], f32)
    Wo = np.asarray(inputs["Wo"], f32)
    bo = np.asarray(inputs["bo"], f32)
    Wg = np.asarray(inputs["Wg"], f32)
    bg = np.asarray(inputs["bg"], f32)
    Wn = np.asarray(inputs["Wn"], f32)
    bn = np.asarray(inputs["bn"], f32)
    We = np.asarray(inputs["We"], f32)
    be = np.asarray(inputs["be"], f32)
    Vw = np.asarray(inputs["Vw"], f32)
    Vb = np.asarray(inputs["Vb"], f32)
    W2w = np.asarray(inputs["W2w"], f32)
    W2b = np.asarray(inputs["W2b"], f32)

    # Fold LN gains/biases into the consuming weights (exact when g=1, b=0).
    triv1 = np.all(ln1_g == 1.0) and np.all(ln1_b == 0.0)
    if not triv1:
        bq = bq + ln1_b @ Wq
        bk = bk + ln1_b @ Wk
        bv = bv + ln1_b @ Wv
        Wq = ln1_g[:, None] * Wq
        Wk = ln1_g[:, None] * Wk
        Wv = ln1_g[:, None] * Wv
    triv2 = np.all(ln2_g == 1.0) and np.all(ln2_b == 0.0)
    if not triv2:
        bg = bg + ln2_b @ Wg
        bn = bn + ln2_b @ Wn
        Vb = Vb + ln2_b @ Vw
        be = be + np.einsum("m,emd->ed", ln2_b, We).astype(f32)
        Wg = ln2_g[:, None] * Wg
        Wn = ln2_g[:, None] * Wn
        Vw = ln2_g[:, None] * Vw
        We = ln2_g[None, :, None] * We
    flags = (bool(np.any(bq)), bool(np.any(bk)), bool(np.any(bv)),
             bool(np.any(bg)), bool(np.any(bn)), bool(np.any(be)),
             bool(np.any(Vb)), bool(np.any(W2b)))
    nc = _get_program(flags)

    we_b = np.ascontiguousarray(We.astype(bf))
    vw_b = np.ascontiguousarray(Vw.astype(bf))
    w2_b = np.ascontiguousarray(W2w.astype(bf))
    wq_c = np.ascontiguousarray(Wq)
    wk_c = np.ascontiguousarray(Wk)
    wv_c = np.ascontiguousarray(Wv)
    wo_c = np.ascontiguousarray(Wo)
    wg_c = np.ascontiguousarray(Wg)
    wn_c = np.ascontiguousarray(Wn)

    shared = {"wq": wq_c, "wk": wk_c, "wv": wv_c, "wo": wo_c,
              "wg": wg_c, "wn": wn_c, "we": we_b, "vw": vw_b, "w2": w2_b}
    if flags[0]:
        shared["bqT"] = np.ascontiguousarray(bq.reshape(MT, P).T.astype(f32))
    if flags[1]:
        shared["bkT"] = np.ascontiguousarray(bk.reshape(KV // P, P).T.astype(f32))
    if flags[2]:
        shared["bvr"] = np.ascontiguousarray(
            np.broadcast_to(bv, (P, KV)).astype(f32))
    if flags[3]:
        shared["bgr"] = np.ascontiguousarray(
            np.broadcast_to(bg, (P, E)).astype(f32))
    if flags[4]:
        shared["bnr"] = np.ascontiguousarray(
            np.broadcast_to(bn, (P, E)).astype(f32))
    if flags[5]:
        shared["ber"] = np.ascontiguousarray(be.astype(f32))
    if flags[6]:
        shared["vbr"] = np.ascontiguousarray(
            np.broadcast_to(Vb, (P, DH)).astype(f32))
    if flags[7]:
        shared["w2br"] = np.ascontiguousarray(
            np.broadcast_to(W2b, (P, M)).astype(f32))

    in_maps = []
    for c in range(8):
        b = c // 4
        r = c % 4
        xb_c = np.ascontiguousarray(np.roll(x[b], -TL * r, axis=0))
        m = dict(shared)
        m["xb"] = xb_c
        m["xpb"] = np.ascontiguousarray(xb_c[:TL] + bo)
        m["noise"] = np.ascontiguousarray(noise[c * TL:(c + 1) * TL])
        in_maps.append(m)

    trace = os.environ.get("KERNEL_TRACE") == "1"
    res = run_bass_kernel_spmd(nc, in_maps, core_ids=list(range(8)),
                               trace=trace)
    kernel.last_results = res

    out = np.empty((2048, M), dtype=f32)
    u_full = np.empty((2048, E), dtype=f32)
    gates_full = np.empty((2048, E), dtype=f32)
    for c in range(8):
        out[c * TL:(c + 1) * TL] = res.results[c]["out"]
        u_full[c * TL:(c + 1) * TL] = res.results[c]["u"]
        gates_full[c * TL:(c + 1) * TL] = res.results[c]["gates"]

    from scipy.special import ndtr

    def _cv(v):
        v = v.astype(np.float64)
        return v.std() / (v.mean() + 1e-6)

    Pm = ndtr(u_full.astype(np.float64))
    loss = f32(0.01 * _cv(gates_full.sum(0)) + 0.01 * _cv(Pm.sum(0)))
    return (out.reshape(2, 1024, M), loss)


# revision 19
# speedup vs baseline: 1.0971x; 1.0971x over previous
"""Trainium2 Bass kernel for nn_MoETransformerEncoderLayer_52750788329547.

Sharding: token-parallel across 8 NeuronCores. Each core owns 256 tokens
(batch c//4, row block c%4), runs LN1 + GQA attention against its full batch
(keys/values recomputed locally), LN2, noisy-top-2 gating, and the dense
8-expert MoE combine for its tokens. No device collectives. The two scalar
aux-loss reductions (E=8 column sums) are finished on the host from per-core
(256,8) outputs.

Precision: everything that feeds the top-k routing (attention -> LN2 ->
gating logits / noise scale) is computed in fp32 (min top-2/3 gap in Hn is
~4e-5, so bf16 there would flip expert routing vs the reference). The expert
matmuls / xV / W2 run in bf16 with fp32 PSUM accumulation.

Per-core layouts place tokens on SBUF partitions for LN/softmax-denominator/
gating reductions, and features on partitions for matmul stationary operands;
PE transposes (via identity) bridge the two.
"""
import os
import sys

sys.path.insert(0, "/opt/trn_rl_repo")

import numpy as np
import ml_dtypes

import concourse.bass as bass
import concourse.mybir as mybir
from concourse import bacc, tile
from concourse.bass_utils import run_bass_kernel_spmd
from concourse.masks import make_identity

dt = mybir.dt
AF = mybir.ActivationFunctionType
OP = mybir.AluOpType
AX = mybir.AxisListType

P = 128
M = 1024          # model dim
DH = 4096         # expert hidden dim
E = 8             # experts
NH = 16           # heads
NG = 4            # kv groups
HD = 64           # head dim
KV = NG * HD      # 256
NB = 1024         # tokens per batch
TL = 256          # tokens per core
SCALE = HD ** -0.5
BIG = 1e30
EPS = 1e-5

MT = M // P       # 8 m-tiles
TT = TL // P      # 2 local token tiles
NT = NB // P      # 8 batch token tiles
DC = DH // 512    # 8 dh chunks of 512
DT = DH // P      # 32 dh tiles of 128

_programs = {}

# float32r measured at rel~1.6e-4 on HW — too coarse for the routing chain
# (min top-k gap ~4e-5), so plain fp32 matmuls are the default there.
USE_F32R = os.environ.get("KERNEL_F32R", "0") == "1"


def _ln_stats(nc, pool, xt_ap, width):
    """Return (rsig, nmr) [P,1] f32 tiles: y = x*rsig + nmr is LayerNorm(x)
    (gain/bias folded into downstream weights). Newton-refined rsqrt."""
    ssum = pool.tile([P, 1], dt.float32, tag="ssum", name="ssum")
    nc.vector.reduce_sum(ssum[:], xt_ap, axis=AX.X)
    sq = pool.tile([P, width], dt.float32, tag="sqscratch", name="sq")
    ssq = pool.tile([P, 1], dt.float32, tag="ssq", name="ssq")
    nc.scalar.activation(sq[:], xt_ap, AF.Square, accum_out=ssq[:])
    mu = pool.tile([P, 1], dt.float32, tag="mu", name="mu")
    nc.vector.tensor_scalar_mul(mu[:], ssum[:], 1.0 / width)
    ex2 = pool.tile([P, 1], dt.float32, tag="ex2", name="ex2")
    nc.vector.tensor_scalar_mul(ex2[:], ssq[:], 1.0 / width)
    mu2 = pool.tile([P, 1], dt.float32, tag="mu2", name="mu2")
    nc.vector.tensor_mul(mu2[:], mu[:], mu[:])
    ve = pool.tile([P, 1], dt.float32, tag="ve", name="ve")
    nc.vector.tensor_sub(ve[:], ex2[:], mu2[:])
    nc.vector.tensor_scalar_add(ve[:], ve[:], EPS)
    s0 = pool.tile([P, 1], dt.float32, tag="s0", name="s0")
    nc.scalar.activation(s0[:], ve[:], AF.Sqrt)
    r0 = pool.tile([P, 1], dt.float32, tag="r0", name="r0")
    nc.vector.reciprocal(r0[:], s0[:])
    vr = pool.tile([P, 1], dt.float32, tag="vr", name="vr")
    nc.vector.tensor_mul(vr[:], ve[:], r0[:])
    s1h = pool.tile([P, 1], dt.float32, tag="s1h", name="s1h")
    nc.vector.tensor_add(s1h[:], s0[:], vr[:])  # = 2*sqrt(ve) after Newton
    rsig = pool.tile([P, 1], dt.float32, tag="rsig", name="rsig")
    nc.vector.reciprocal(rsig[:], s1h[:])
    nc.vector.tensor_scalar_mul(rsig[:], rsig[:], 2.0)
    nmr = pool.tile([P, 1], dt.float32, tag="nmr", name="nmr")
    nc.vector.tensor_scalar(nmr[:], mu[:], rsig[:], -1.0, OP.mult, OP.mult)
    return rsig, nmr


def _build_program(flags):
    has_bq, has_bk, has_bv, has_bg, has_bn, has_be, has_vb, has_w2b = flags
    f32 = dt.float32
    bf16 = dt.bfloat16
    fr = dt.float32r if USE_F32R else dt.float32

    nc = bacc.Bacc("TRN2", target_bir_lowering=False, debug=False,
                   num_devices=8)

    # ---- I/O ----
    # Weights arrive pre-tiled (host rearrange) so every DMA below reads one
    # fully-contiguous DRAM block — avoids the 512B-per-packet strided reads
    # that made the first version DMA-bound.
    xb = nc.dram_tensor("xb", [NB, M], f32, kind="ExternalInput")
    xpb = nc.dram_tensor("xpb", [TL, M], f32, kind="ExternalInput")
    noise_d = nc.dram_tensor("noise", [TL, E], f32, kind="ExternalInput")
    wq_d = nc.dram_tensor("wq", [MT, MT, P, P], f32, kind="ExternalInput")
    wk_d = nc.dram_tensor("wk", [KV // P, MT, P, P], f32, kind="ExternalInput")
    wv_d = nc.dram_tensor("wv", [M, KV], f32, kind="ExternalInput")
    wo_d = nc.dram_tensor("wo", [2, MT, P, 512], f32, kind="ExternalInput")
    wg_d = nc.dram_tensor("wg", [M, E], f32, kind="ExternalInput")
    wn_d = nc.dram_tensor("wn", [M, E], f32, kind="ExternalInput")
    we_d = nc.dram_tensor("we", [E, 2, MT, P, DH // 2], bf16,
                          kind="ExternalInput")
    vw_d = nc.dram_tensor("vw", [2, MT, P, DH // 2], bf16,
                          kind="ExternalInput")
    w2_d = nc.dram_tensor("w2", [DH, M], bf16, kind="ExternalInput")
    bqT_d = bkT_d = bvr_d = bgr_d = bnr_d = ber_d = vbr_d = w2br_d = None
    if has_bq:
        bqT_d = nc.dram_tensor("bqT", [P, MT], f32, kind="ExternalInput")
    if has_bk:
        bkT_d = nc.dram_tensor("bkT", [P, KV // P], f32, kind="ExternalInput")
    if has_bv:
        bvr_d = nc.dram_tensor("bvr", [P, KV], f32, kind="ExternalInput")
    if has_bg:
        bgr_d = nc.dram_tensor("bgr", [P, E], f32, kind="ExternalInput")
    if has_bn:
        bnr_d = nc.dram_tensor("bnr", [P, E], f32, kind="ExternalInput")
    if has_be:
        ber_d = nc.dram_tensor("ber", [E, DH], f32, kind="ExternalInput")
    if has_vb:
        vbr_d = nc.dram_tensor("vbr", [P, DH], f32, kind="ExternalInput")
    if has_w2b:
        w2br_d = nc.dram_tensor("w2br", [P, M], f32, kind="ExternalInput")

    out_d = nc.dram_tensor("out", [TL, M], f32, kind="ExternalOutput")
    u_d = nc.dram_tensor("u", [TL, E], f32, kind="ExternalOutput")
    gates_d = nc.dram_tensor("gates", [TL, E], f32, kind="ExternalOutput")

    from contextlib import ExitStack

    with tile.TileContext(nc) as tc, ExitStack() as top:
        const = top.enter_context(tc.tile_pool(name="const", bufs=1))
        id32 = const.tile([P, P], f32, name="id32")
        make_identity(nc, id32[:])
        idb = const.tile([P, P], bf16, name="idb")
        make_identity(nc, idb[:])

        # persistent activation buffers
        persist = top.enter_context(tc.tile_pool(name="persist", bufs=1))
        x2_all = persist.tile([P, TT * M], f32, name="x2_all")
        xpb_all = persist.tile([P, TT * M], f32, name="xpb_all")
        for t in range(TT):
            nc.sync.dma_start(xpb_all[:, t * M:(t + 1) * M],
                              xpb[t * P:(t + 1) * P, :])

        # ============ Phase 1: LN1 over the full batch ============
        with ExitStack() as ph:
            ypool = ph.enter_context(tc.tile_pool(name="ypool", bufs=1))
            y_all = ypool.tile([P, NT * M], f32, name="y_all")
            with ExitStack() as inner:
                xin = inner.enter_context(tc.tile_pool(name="xin", bufs=3))
                lns = inner.enter_context(tc.tile_pool(name="lns", bufs=3))
                for t in range(NT):
                    xt = xin.tile([P, M], f32, tag="xt", name="xt")
                    nc.sync.dma_start(xt[:], xb[t * P:(t + 1) * P, :])
                    rsig, nmr = _ln_stats(nc, lns, xt[:], M)
                    nc.scalar.activation(y_all[:, t * M:(t + 1) * M], xt[:],
                                         AF.Identity, bias=nmr[:],
                                         scale=rsig[:])

            # ============ Phase 2: y^T ============
            ytp = ph.enter_context(tc.tile_pool(name="ytp", bufs=1))
            yT = ytp.tile([P, NT * M], fr, name="yT")
            with ExitStack() as inner:
                tps = inner.enter_context(
                    tc.tile_pool(name="tps", bufs=4, space="PSUM"))
                for t in range(NT):
                    for mt in range(MT):
                        pst = tps.tile([P, P], f32, tag="pst", name="pst")
                        nc.tensor.transpose(
                            pst[:], y_all[:, t * M + mt * P: t * M + mt * P + P],
                            id32[:])
                        nc.scalar.copy(
                            yT[:, mt * NB + t * P: mt * NB + t * P + P], pst[:])

            # ============ Phase 3: q^T, k^T, v ============
            attn = ph.enter_context(tc.tile_pool(name="attn", bufs=1))
            qT = attn.tile([P, MT * TL], fr, name="qT")
            kT = attn.tile([P, (KV // P) * NB], fr, name="kT")
            v_all = attn.tile([P, NT * (NG * (HD + 1))], fr, name="v_all")
            GW = NG * (HD + 1)  # 260 columns per key tile

            with ExitStack() as inner:
                wst = inner.enter_context(tc.tile_pool(name="wst", bufs=4))
                qps = inner.enter_context(
                    tc.tile_pool(name="qps", bufs=2, space="PSUM"))
                bq_sb = None
                if has_bq:
                    bq_sb = attn.tile([P, MT], f32, name="bq_sb")
                    nc.sync.dma_start(bq_sb[:], bqT_d[:, :])
                bk_sb = None
                if has_bk:
                    bk_sb = attn.tile([P, KV // P], f32, name="bk_sb")
                    nc.sync.dma_start(bk_sb[:], bkT_d[:, :])
                bv_sb = None
                if has_bv:
                    bv_sb = attn.tile([P, KV], f32, name="bv_sb")
                    nc.sync.dma_start(bv_sb[:], bvr_d[:, :])

                def wtile(dram_ap, tag):
                    """DMA a weight tile; convert to f32r via DVE if needed."""
                    t0 = wst.tile([P, dram_ap.shape[-1]], f32, tag=tag,
                                  name=tag)
                    nc.sync.dma_start(t0[:], dram_ap)
                    if not USE_F32R:
                        return t0
                    t1 = wst.tile([P, dram_ap.shape[-1]], fr, tag=tag + "r",
                                  name=tag + "r")
                    nc.vector.tensor_copy(t1[:], t0[:])
                    return t1

                # q^T (features on partitions, local 256 tokens on free)
                for ht in range(MT):
                    psq = qps.tile([P, TL], f32, tag="psq", name="psq")
                    for mt in range(MT):
                        wt = wtile(wq_d[ht, mt], "wq")
                        nc.tensor.matmul(psq[:], wt[:],
                                         yT[:, mt * NB: mt * NB + TL],
                                         start=(mt == 0), stop=(mt == MT - 1))
                    if has_bq:
                        nc.scalar.activation(qT[:, ht * TL:(ht + 1) * TL],
                                             psq[:], AF.Identity,
                                             bias=bq_sb[:, ht:ht + 1])
                    else:
                        nc.scalar.copy(qT[:, ht * TL:(ht + 1) * TL], psq[:])

                # k^T (kv features on partitions, all 1024 batch tokens free)
                for kt in range(KV // P):
                    psk = [qps.tile([P, 512], f32, tag=f"psk{ch}",
                                    name=f"psk{ch}") for ch in range(2)]
                    for mt in range(MT):
                        wt = wtile(wk_d[kt, mt], "wk")
                        for ch in range(2):
                            nc.tensor.matmul(
                                psk[ch][:], wt[:],
                                yT[:, mt * NB + ch * 512: mt * NB + ch * 512 + 512],
                                start=(mt == 0), stop=(mt == MT - 1))
                    for ch in range(2):
                        dst = kT[:, kt * NB + ch * 512: kt * NB + ch * 512 + 512]
                        if has_bk:
                            nc.scalar.activation(dst, psk[ch][:], AF.Identity,
                                                 bias=bk_sb[:, kt:kt + 1])
                        else:
                            nc.scalar.copy(dst, psk[ch][:])

                # v natural (batch tokens on partitions) with ones column per group
                wv_sb = attn.tile([P, MT * KV], f32, name="wv_sb")
                for mt in range(MT):
                    nc.sync.dma_start(wv_sb[:, mt * KV:(mt + 1) * KV],
                                      wv_d[mt * P:(mt + 1) * P, :])
                for t in range(NT):
                    psv = qps.tile([P, KV], f32, tag="psv", name="psv")
                    for mt in range(MT):
                        nc.tensor.matmul(
                            psv[:],
                            yT[:, mt * NB + t * P: mt * NB + t * P + P],
                            wv_sb[:, mt * KV:(mt + 1) * KV],
                            start=(mt == 0), stop=(mt == MT - 1))
                    base = t * GW
                    nc.vector.memset(v_all[:, base: base + GW], 1.0)
                    for g in range(NG):
                        dst = v_all[:, base + g * (HD + 1): base + g * (HD + 1) + HD]
                        if has_bv:
                            nc.vector.scalar_tensor_tensor(
                                dst, psv[:, g * HD:(g + 1) * HD], 1.0,
                                bv_sb[:, g * HD:(g + 1) * HD], OP.mult, OP.add)
                        else:
                            nc.vector.tensor_copy(dst, psv[:, g * HD:(g + 1) * HD])

            # ============ Phase 4: attention heads ============
            aT = attn.tile([P, MT * TL], fr, name="aT")
            with ExitStack() as inner:
                ptp = inner.enter_context(tc.tile_pool(name="ptp", bufs=2))
                sps = inner.enter_context(
                    tc.tile_pool(name="sps", bufs=3, space="PSUM"))
                aps = inner.enter_context(
                    tc.tile_pool(name="aps", bufs=2, space="PSUM"))
                hsm = inner.enter_context(tc.tile_pool(name="hsm", bufs=2))
                for h in range(NH):
                    g = h % NG
                    krow = (g % 2) * HD
                    kcol = (g // 2) * NB
                    qrow = (h % 2) * HD
                    qcol = (h // 2) * TL
                    PTt = ptp.tile([P, NT * TL], fr, tag="PT", name="PTt")
                    for k8 in range(NT):
                        pss = sps.tile([P, TL], f32, tag="pss", name="pss")
                        nc.tensor.matmul(
                            pss[:],
                            kT[krow:krow + HD, kcol + k8 * P: kcol + k8 * P + P],
                            qT[qrow:qrow + HD, qcol: qcol + TL],
                            start=True, stop=True)
                        nc.scalar.activation(PTt[:, k8 * TL:(k8 + 1) * TL],
                                             pss[:], AF.Exp, scale=SCALE)
                    psa = aps.tile([HD + 1, TL], f32, tag="psa", name="psa")
                    for k8 in range(NT):
                        nc.tensor.matmul(
                            psa[:],
                            v_all[:, k8 * GW + g * (HD + 1): k8 * GW + (g + 1) * (HD + 1)],
                            PTt[:, k8 * TL:(k8 + 1) * TL],
                            start=(k8 == 0), stop=(k8 == NT - 1))
                    rr = hsm.tile([1, TL], f32, tag="rr", name="rr")
                    nc.vector.reciprocal(rr[:], psa[HD:HD + 1, :])
                    rb = hsm.tile([HD, TL], f32, tag="rb", name="rb")
                    nc.gpsimd.partition_broadcast(rb[:], rr[:])
                    nc.vector.tensor_mul(
                        aT[qrow:qrow + HD, qcol: qcol + TL],
                        psa[0:HD, :], rb[:])

            # ============ Phase 5: Wo + residual ============
            with ExitStack() as inner:
                wst = inner.enter_context(tc.tile_pool(name="wst2", bufs=4))
                wps = inner.enter_context(
                    tc.tile_pool(name="wps", bufs=4, space="PSUM"))
                for mc in range(2):
                    psw = [wps.tile([P, 512], f32, tag=f"psw{t}",
                                    name=f"psw{t}") for t in range(TT)]
                    for kt in range(MT):
                        t0 = wst.tile([P, 512], f32, tag="wo", name="wo_t")
                        nc.sync.dma_start(t0[:], wo_d[mc, kt])
                        if USE_F32R:
                            wt = wst.tile([P, 512], fr, tag="wor", name="wor_t")
                            nc.vector.tensor_copy(wt[:], t0[:])
                        else:
                            wt = t0
                        for t in range(TT):
                            nc.tensor.matmul(
                                psw[t][:],
                                aT[:, kt * TL + t * P: kt * TL + t * P + P],
                                wt[:], start=(kt == 0), stop=(kt == MT - 1))
                    for t in range(TT):
                        nc.vector.tensor_add(
                            x2_all[:, t * M + mc * 512: t * M + mc * 512 + 512],
                            psw[t][:],
                            xpb_all[:, t * M + mc * 512: t * M + mc * 512 + 512])

        # ============ Phase 6: LN2 -> z, z^T ============
        moe = top.enter_context(tc.tile_pool(name="moe", bufs=1))
        zT32 = moe.tile([P, MT * TL], f32, name="zT32")
        zTb = moe.tile([P, MT * TL], bf16, name="zTb")
        with ExitStack() as inner:
            lns = inner.enter_context(tc.tile_pool(name="lns2", bufs=2))
            zp = inner.enter_context(tc.tile_pool(name="zp", bufs=1))
            z_all = zp.tile([P, TT * M], f32, name="z_all")
            for t in range(TT):
                rsig, nmr = _ln_stats(nc, lns, x2_all[:, t * M:(t + 1) * M], M)
                nc.scalar.activation(z_all[:, t * M:(t + 1) * M],
                                     x2_all[:, t * M:(t + 1) * M],
                                     AF.Identity, bias=nmr[:], scale=rsig[:])
            tps = inner.enter_context(
                tc.tile_pool(name="tps2", bufs=4, space="PSUM"))
            for t in range(TT):
                for mt in range(MT):
                    pst = tps.tile([P, P], f32, tag="pst", name="pst2")
                    nc.tensor.transpose(
                        pst[:], z_all[:, t * M + mt * P: t * M + mt * P + P],
                        id32[:])
                    nc.scalar.copy(
                        zT32[:, mt * TL + t * P: mt * TL + t * P + P], pst[:])
                    nc.vector.tensor_copy(
                        zTb[:, mt * TL + t * P: mt * TL + t * P + P], pst[:])

        # ============ Phase 7: gating ============
        gates_all = moe.tile([P, TT * E], f32, name="gates_all")
        with ExitStack() as inner:
            gsb = inner.enter_context(tc.tile_pool(name="gsb", bufs=2))
            gps = inner.enter_context(
                tc.tile_pool(name="gps", bufs=2, space="PSUM"))
            wg_sb = gsb.tile([P, MT * E], f32, tag="wg", name="wg_sb")
            wn_sb = gsb.tile([P, MT * E], f32, tag="wn", name="wn_sb")
            for mt in range(MT):
                nc.sync.dma_start(wg_sb[:, mt * E:(mt + 1) * E],
                                  wg_d[mt * P:(mt + 1) * P, :])
                nc.sync.dma_start(wn_sb[:, mt * E:(mt + 1) * E],
                                  wn_d[mt * P:(mt + 1) * P, :])
            bg_sb = bn_sb = None
            if has_bg:
                bg_sb = gsb.tile([P, E], f32, tag="bg", name="bg_sb")
                nc.sync.dma_start(bg_sb[:], bgr_d[:, :])
            if has_bn:
                bn_sb = gsb.tile([P, E], f32, tag="bn", name="bn_sb")
                nc.sync.dma_start(bn_sb[:], bnr_d[:, :])

            for t in range(TT):
                psl = gps.tile([P, E], f32, tag="psl", name="psl")
                psn = gps.tile([P, E], f32, tag="psn", name="psn")
                for mt in range(MT):
                    lhsT = zT32[:, mt * TL + t * P: mt * TL + t * P + P]
                    nc.tensor.matmul(psl[:], lhsT, wg_sb[:, mt * E:(mt + 1) * E],
                                     start=(mt == 0), stop=(mt == MT - 1))
                    nc.tensor.matmul(psn[:], lhsT, wn_sb[:, mt * E:(mt + 1) * E],
                                     start=(mt == 0), stop=(mt == MT - 1))

                def gt(tag):
                    return gsb.tile([P, E], f32, tag=tag, name=tag)

                logits = gt("logits")
                if has_bg:
                    nc.vector.tensor_add(logits[:], psl[:], bg_sb[:])
                else:
                    nc.scalar.copy(logits[:], psl[:])
                raw = gt("raw")
                if has_bn:
                    nc.vector.tensor_add(raw[:], psn[:], bn_sb[:])
                else:
                    nc.scalar.copy(raw[:], psn[:])
                # ns = softplus(raw) = relu(raw) + ln(1 + exp(-|raw|))
                t_abs = gt("t_abs")
                nc.scalar.activation(t_abs[:], raw[:], AF.Abs)
                t_exp = gt("t_exp")
                nc.scalar.activation(t_exp[:], t_abs[:], AF.Exp, scale=-1.0)
                t_ln = gt("t_ln")
                nc.scalar.activation(t_ln[:], t_exp[:], AF.Ln, bias=1.0)
                t_rel = gt("t_rel")
                nc.scalar.activation(t_rel[:], raw[:], AF.Relu)
                ns = gt("ns")
                nc.vector.tensor_add(ns[:], t_rel[:], t_ln[:])
                noise_sb = gt("noise_sb")
                nc.sync.dma_start(noise_sb[:], noise_d[t * P:(t + 1) * P, :])
                Hn = gt("Hn")
                nc.vector.tensor_mul(Hn[:], noise_sb[:], ns[:])
                nc.vector.tensor_add(Hn[:], Hn[:], logits[:])

                def col(tag):
                    return gsb.tile([P, 1], f32, tag=tag, name=tag)

                m1 = col("m1")
                nc.vector.reduce_max(m1[:], Hn[:], axis=AX.X)
                mask1 = gt("mask1")
                nc.vector.tensor_scalar(mask1[:], Hn[:], m1[:], None, OP.is_ge)
                Hn2 = gt("Hn2")
                nc.vector.scalar_tensor_tensor(Hn2[:], mask1[:], -BIG, Hn[:],
                                               OP.mult, OP.add)
                m2 = col("m2")
                nc.vector.reduce_max(m2[:], Hn2[:], axis=AX.X)
                mask2 = gt("mask2")
                nc.vector.tensor_scalar(mask2[:], Hn2[:], m2[:], None, OP.is_ge)
                Hn3 = gt("Hn3")
                nc.vector.scalar_tensor_tensor(Hn3[:], mask2[:], -BIG, Hn2[:],
                                               OP.mult, OP.add)
                m3 = col("m3")
                nc.vector.reduce_max(m3[:], Hn3[:], axis=AX.X)
                # gates = mask1*sig(m1-m2) + mask2*(1-sig(m1-m2))
                d = col("d")
                nc.vector.tensor_sub(d[:], m2[:], m1[:])
                ed = col("ed")
                nc.scalar.activation(ed[:], d[:], AF.Exp)
                nc.vector.tensor_scalar_add(ed[:], ed[:], 1.0)
                g1 = col("g1")
                nc.vector.reciprocal(g1[:], ed[:])
                g2 = col("g2")
                nc.scalar.activation(g2[:], g1[:], AF.Copy, bias=1.0,
                                     scale=-1.0)
                t1g = gt("t1g")
                nc.vector.tensor_scalar(t1g[:], mask1[:], g1[:], None, OP.mult)
                nc.vector.scalar_tensor_tensor(
                    gates_all[:, t * E:(t + 1) * E], mask2[:], g2[:], t1g[:],
                    OP.mult, OP.add)
                # psi = m3 + (mask1+mask2)*(m2-m3); u = (logits-psi)/ns
                msum = gt("msum")
                nc.vector.tensor_add(msum[:], mask1[:], mask2[:])
                d23 = col("d23")
                nc.vector.tensor_sub(d23[:], m2[:], m3[:])
                psi = gt("psi")
                nc.vector.tensor_scalar(psi[:], msum[:], d23[:], m3[:],
                                        OP.mult, OP.add)
                lp = gt("lp")
                nc.vector.tensor_sub(lp[:], logits[:], psi[:])
                rns = gt("rns")
                nc.vector.reciprocal(rns[:], ns[:])
                uu = gt("uu")
                nc.vector.tensor_mul(uu[:], lp[:], rns[:])
                nc.sync.dma_start(u_d[t * P:(t + 1) * P, :], uu[:])
                nc.sync.dma_start(gates_d[t * P:(t + 1) * P, :],
                                  gates_all[:, t * E:(t + 1) * E])

        # ============ Phase 8: xV ============
        s_all = moe.tile([P, TT * DH], f32, name="s_all")
        xv_all = moe.tile([P, TT * DH], bf16, name="xv_all")
        nc.gpsimd.memset(s_all[:], 0.0)
        with ExitStack() as inner:
            wst = inner.enter_context(tc.tile_pool(name="wst3", bufs=4))
            xps = inner.enter_context(
                tc.tile_pool(name="xps", bufs=1, space="PSUM"))
            vb_sb = None
            if has_vb:
                vbp = inner.enter_context(tc.tile_pool(name="vbp", bufs=1))
                vb_sb = vbp.tile([P, DH], f32, name="vb_sb")
                nc.sync.dma_start(vb_sb[:], vbr_d[:, :])
            for half in range(2):
                psx = {(dcl, t): xps.tile([P, 512], f32, tag=f"psx{dcl}_{t}",
                                          name=f"psx{dcl}_{t}")
                       for dcl in range(4) for t in range(TT)}
                for mt in range(MT):
                    vt = wst.tile([P, DH // 2], bf16, tag="vwt", name="vwt")
                    nc.sync.dma_start(vt[:], vw_d[half, mt])
                    for dcl in range(4):
                        for t in range(TT):
                            nc.tensor.matmul(
                                psx[dcl, t][:],
                                zTb[:, mt * TL + t * P: mt * TL + t * P + P],
                                vt[:, dcl * 512:(dcl + 1) * 512],
                                start=(mt == 0), stop=(mt == MT - 1))
                for dcl in range(4):
                    dc = half * 4 + dcl
                    for t in range(TT):
                        dst = xv_all[:, t * DH + dc * 512: t * DH + dc * 512 + 512]
                        if has_vb:
                            nc.vector.tensor_add(
                                dst, psx[dcl, t][:],
                                vb_sb[:, dc * 512:(dc + 1) * 512])
                        else:
                            nc.scalar.copy(dst, psx[dcl, t][:])

        # ============ Phase 9: experts (dense, bf16) ============
        with ExitStack() as inner:
            wst = inner.enter_context(tc.tile_pool(name="wst4", bufs=3))
            eps_ = inner.enter_context(
                tc.tile_pool(name="eps", bufs=1, space="PSUM"))
            spool = inner.enter_context(tc.tile_pool(name="spool", bufs=3))
            be_sb = None
            if has_be:
                bep = inner.enter_context(tc.tile_pool(name="bep", bufs=2))
            for e in range(E):
                if has_be:
                    ber_row = bep.tile([1, DH], f32, tag="ber_row",
                                       name="ber_row")
                    nc.sync.dma_start(ber_row[:], ber_d[e:e + 1, :])
                    be_sb = bep.tile([P, DH], f32, tag="be_sb", name="be_sb")
                    nc.gpsimd.partition_broadcast(be_sb[:], ber_row[:])
                for half in range(2):
                    pse = {(dcl, t): eps_.tile([P, 512], f32,
                                               tag=f"pse{dcl}_{t}",
                                               name=f"pse{dcl}_{t}")
                           for dcl in range(4) for t in range(TT)}
                    for mt in range(MT):
                        wt = wst.tile([P, DH // 2], bf16, tag="wet",
                                      name="wet")
                        nc.sync.dma_start(wt[:], we_d[e, half, mt])
                        for dcl in range(4):
                            for t in range(TT):
                                nc.tensor.matmul(
                                    pse[dcl, t][:],
                                    zTb[:, mt * TL + t * P: mt * TL + t * P + P],
                                    wt[:, dcl * 512:(dcl + 1) * 512],
                                    start=(mt == 0), stop=(mt == MT - 1))
                    for dcl in range(4):
                        dc = half * 4 + dcl
                        for t in range(TT):
                            if has_be:
                                nc.vector.tensor_add(
                                    pse[dcl, t][:], pse[dcl, t][:],
                                    be_sb[:, dc * 512:(dc + 1) * 512])
                            sil = spool.tile([P, 512], bf16, tag="sil",
                                             name="sil")
                            nc.scalar.activation(sil[:], pse[dcl, t][:],
                                                 AF.Silu)
                            sl = s_all[:, t * DH + dc * 512:
                                       t * DH + dc * 512 + 512]
                            nc.vector.scalar_tensor_tensor(
                                sl, sil[:],
                                gates_all[:, t * E + e: t * E + e + 1],
                                sl, OP.mult, OP.add)

        # ============ Phase 10: gated, gated^T ============
        gT = moe.tile([P, DT * TL], bf16, name="gT")
        with ExitStack() as inner:
            gp = inner.enter_context(tc.tile_pool(name="gp", bufs=2))
            tps = inner.enter_context(
                tc.tile_pool(name="tps3", bufs=4, space="PSUM"))
            for t in range(TT):
                gtmp = gp.tile([P, DH], bf16, tag="gtmp", name="gtmp")
                nc.vector.tensor_mul(gtmp[:], s_all[:, t * DH:(t + 1) * DH],
                                     xv_all[:, t * DH:(t + 1) * DH])
                for dtt in range(DT):
                    pst = tps.tile([P, P], bf16, tag="pst", name="pst3")
                    nc.tensor.transpose(pst[:], gtmp[:, dtt * P:(dtt + 1) * P],
                                        idb[:])
                    nc.scalar.copy(
                        gT[:, dtt * TL + t * P: dtt * TL + t * P + P], pst[:])

        # ============ Phase 11: W2 + residual out ============
        with ExitStack() as inner:
            wst = inner.enter_context(tc.tile_pool(name="wst5", bufs=4))
            fps = inner.enter_context(
                tc.tile_pool(name="fps", bufs=1, space="PSUM"))
            opool = inner.enter_context(tc.tile_pool(name="opool", bufs=3))
            w2b_sb = None
            if has_w2b:
                wbp = inner.enter_context(tc.tile_pool(name="wbp", bufs=1))
                w2b_sb = wbp.tile([P, M], f32, name="w2b_sb")
                nc.sync.dma_start(w2b_sb[:], w2br_d[:, :])
            psf = {(mc, t): fps.tile([P, 512], f32, tag=f"psf{mc}_{t}",
                                     name=f"psf{mc}_{t}")
                   for mc in range(2) for t in range(TT)}
            for dtt in range(DT):
                w2t = wst.tile([P, M], bf16, tag="w2t", name="w2t")
                nc.sync.dma_start(w2t[:], w2_d[dtt * P:(dtt + 1) * P, :])
                for mc in range(2):
                    for t in range(TT):
                        nc.tensor.matmul(
                            psf[mc, t][:],
                            gT[:, dtt * TL + t * P: dtt * TL + t * P + P],
                            w2t[:, mc * 512:(mc + 1) * 512],
                            start=(dtt == 0), stop=(dtt == DT - 1))
            for mc in range(2):
                for t in range(TT):
                    ot = opool.tile([P, 512], f32, tag="ot", name="ot")
                    nc.vector.tensor_add(
                        ot[:], psf[mc, t][:],
                        x2_all[:, t * M + mc * 512: t * M + mc * 512 + 512])
                    if has_w2b:
                        nc.vector.tensor_add(
                            ot[:], ot[:], w2b_sb[:, mc * 512:(mc + 1) * 512])
                    nc.sync.dma_start(
                        out_d[t * P:(t + 1) * P, mc * 512:(mc + 1) * 512],
                        ot[:])

    nc.compile()
    return nc


def _get_program(flags):
    if flags not in _programs:
        _programs[flags] = _build_program(flags)
    return _programs[flags]


def kernel(**inputs):
    f32 = np.float32
    bf = ml_dtypes.bfloat16
    x = np.ascontiguousarray(np.asarray(inputs["x"], dtype=f32))
    noise = np.asarray(inputs["noise"], dtype=f32)
    ln1_g = np.asarray(inputs["ln1_g"], f32)
    ln1_b = np.asarray(inputs["ln1_b"], f32)
    ln2_g = np.asarray(inputs["ln2_g"], f32)
    ln2_b = np.asarray(inputs["ln2_b"], f32)
    Wq = np.asarray(inputs["Wq"], f32)
    bq = np.asarray(inputs["bq"], f32)
    Wk = np.asarray(inputs["Wk"], f32)
    bk = np.asarray(inputs["bk"], f32)
    Wv = np.asarray(inputs["Wv"], f32)
    bv = np.asarray(inputs["bv"], f32)
    Wo = np.asarray(inputs["Wo"], f32)
    bo = np.asarray(inputs["bo"], f32)
    Wg = np.asarray(inputs["Wg"], f32)
    bg = np.asarray(inputs["bg"], f32)
    Wn = np.asarray(inputs["Wn"], f32)
    bn = np.asarray(inputs["bn"], f32)
    We = np.asarray(inputs["We"], f32)
    be = np.asarray(inputs["be"], f32)
    Vw = np.asarray(inputs["Vw"], f32)
    Vb = np.asarray(inputs["Vb"], f32)
    W2w = np.asarray(inputs["W2w"], f32)
    W2b = np.asarray(inputs["W2b"], f32)

    # Fold LN gains/biases into the consuming weights (exact when g=1, b=0).
    triv1 = np.all(ln1_g == 1.0) and np.all(ln1_b == 0.0)
    if not triv1:
        bq = bq + ln1_b @ Wq
        bk = bk + ln1_b @ Wk
        bv = bv + ln1_b @ Wv
        Wq = ln1_g[:, None] * Wq
        Wk = ln1_g[:, None] * Wk
        Wv = ln1_g[:, None] * Wv
    triv2 = np.all(ln2_g == 1.0) and np.all(ln2_b == 0.0)
    if not triv2:
        bg = bg + ln2_b @ Wg
        bn = bn + ln2_b @ Wn
        Vb = Vb + ln2_b @ Vw
        be = be + np.einsum("m,emd->ed", ln2_b, We).astype(f32)
        Wg = ln2_g[:, None] * Wg
        Wn = ln2_g[:, None] * Wn
        Vw = ln2_g[:, None] * Vw
        We = ln2_g[None, :, None] * We
    flags = (bool(np.any(bq)), bool(np.any(bk)), bool(np.any(bv)),
             bool(np.any(bg)), bool(np.any(bn)), bool(np.any(be)),
             bool(np.any(Vb)), bool(np.any(W2b)))
    nc = _get_program(flags)

    # Pre-tile weights so each device DMA reads one contiguous block.
    we_b = np.ascontiguousarray(
        We.astype(bf).reshape(E, MT, P, 2, DH // 2).transpose(0, 3, 1, 2, 4))
    vw_b = np.ascontiguousarray(
        Vw.astype(bf).reshape(MT, P, 2, DH // 2).transpose(2, 0, 1, 3))
    w2_b = np.ascontiguousarray(W2w.astype(bf))
    wq_c = np.ascontiguousarray(
        Wq.reshape(MT, P, MT, P).transpose(2, 0, 1, 3))
    wk_c = np.ascontiguousarray(
        Wk.reshape(MT, P, KV // P, P).transpose(2, 0, 1, 3))
    wv_c = np.ascontiguousarray(Wv)
    wo_c = np.ascontiguousarray(
        Wo.reshape(MT, P, 2, 512).transpose(2, 0, 1, 3))
    wg_c = np.ascontiguousarray(Wg)
    wn_c = np.ascontiguousarray(Wn)

    shared = {"wq": wq_c, "wk": wk_c, "wv": wv_c, "wo": wo_c,
              "wg": wg_c, "wn": wn_c, "we": we_b, "vw": vw_b, "w2": w2_b}
    if flags[0]:
        shared["bqT"] = np.ascontiguousarray(bq.reshape(MT, P).T.astype(f32))
    if flags[1]:
        shared["bkT"] = np.ascontiguousarray(bk.reshape(KV // P, P).T.astype(f32))
    if flags[2]:
        shared["bvr"] = np.ascontiguousarray(
            np.broadcast_to(bv, (P, KV)).astype(f32))
    if flags[3]:
        shared["bgr"] = np.ascontiguousarray(
            np.broadcast_to(bg, (P, E)).astype(f32))
    if flags[4]:
        shared["bnr"] = np.ascontiguousarray(
            np.broadcast_to(bn, (P, E)).astype(f32))
    if flags[5]:
        shared["ber"] = np.ascontiguousarray(be.astype(f32))
    if flags[6]:
        shared["vbr"] = np.ascontiguousarray(
            np.broadcast_to(Vb, (P, DH)).astype(f32))
    if flags[7]:
        shared["w2br"] = np.ascontiguousarray(
            np.broadcast_to(W2b, (P, M)).astype(f32))

    in_maps = []
    for c in range(8):
        b = c // 4
        r = c % 4
        xb_c = np.ascontiguousarray(np.roll(x[b], -TL * r, axis=0))
        m = dict(shared)
        m["xb"] = xb_c
        m["xpb"] = np.ascontiguousarray(xb_c[:TL] + bo)
        m["noise"] = np.ascontiguousarray(noise[c * TL:(c + 1) * TL])
        in_maps.append(m)

    trace = os.environ.get("KERNEL_TRACE") == "1"
    res = run_bass_kernel_spmd(nc, in_maps, core_ids=list(range(8)),
                               trace=trace)
    kernel.last_results = res

    out = np.empty((2048, M), dtype=f32)
    u_full = np.empty((2048, E), dtype=f32)
    gates_full = np.empty((2048, E), dtype=f32)
    for c in range(8):
        out[c * TL:(c + 1) * TL] = res.results[c]["out"]
        u_full[c * TL:(c + 1) * TL] = res.results[c]["u"]
        gates_full[c * TL:(c + 1) * TL] = res.results[c]["gates"]

    from scipy.special import ndtr

    def _cv(v):
        v = v.astype(np.float64)
        return v.std() / (v.mean() + 1e-6)

    Pm = ndtr(u_full.astype(np.float64))
    loss = f32(0.01 * _cv(gates_full.sum(0)) + 0.01 * _cv(Pm.sum(0)))
    return (out.reshape(2, 1024, M), loss)


# revision 26
# speedup vs baseline: 1.2852x; 1.1715x over previous
"""Trainium2 Bass kernel for nn_MoETransformerEncoderLayer_52750788329547.

Sharding: token-parallel across 8 NeuronCores. Each core owns 256 tokens
(batch c//4, row block c%4), runs LN1 + GQA attention against its full batch
(keys/values recomputed locally), LN2, noisy-top-2 gating, and the dense
8-expert MoE combine for its tokens. No device collectives. The two scalar
aux-loss reductions (E=8 column sums) are finished on the host from per-core
(256,8) outputs.

Precision: everything that feeds the top-k routing (attention -> LN2 ->
gating logits / noise scale) is computed in fp32 (min top-2/3 gap in Hn is
~4e-5, so bf16 there would flip expert routing vs the reference). The expert
matmuls / xV / W2 run in bf16 with fp32 PSUM accumulation.

Per-core layouts place tokens on SBUF partitions for LN/softmax-denominator/
gating reductions, and features on partitions for matmul stationary operands;
PE transposes (via identity) bridge the two.
"""
import os
import sys

sys.path.insert(0, "/opt/trn_rl_repo")

import numpy as np
import ml_dtypes

import concourse.bass as bass
import concourse.mybir as mybir
from concourse import bacc, tile
from concourse.bass_utils import run_bass_kernel_spmd
from concourse.masks import make_identity

dt = mybir.dt
AF = mybir.ActivationFunctionType
OP = mybir.AluOpType
AX = mybir.AxisListType

P = 128
M = 1024          # model dim
DH = 4096         # expert hidden dim
E = 8             # experts
NH = 16           # heads
NG = 4            # kv groups
HD = 64           # head dim
KV = NG * HD      # 256
NB = 1024         # tokens per batch
TL = 256          # tokens per core
SCALE = HD ** -0.5
BIG = 1e30
EPS = 1e-5

MT = M // P       # 8 m-tiles
TT = TL // P      # 2 local token tiles
NT = NB // P      # 8 batch token tiles
DC = DH // 512    # 8 dh chunks of 512
DT = DH // P      # 32 dh tiles of 128

_programs = {}

# float32r measured at rel~1.6e-4 on HW — too coarse for the routing chain
# (min top-k gap ~4e-5), so plain fp32 matmuls are the default there.
USE_F32R = os.environ.get("KERNEL_F32R", "0") == "1"


def _ln_stats(nc, pool, xt_ap, width):
    """Return (rsig, nmr) [P,1] f32 tiles: y = x*rsig + nmr is LayerNorm(x)
    (gain/bias folded into downstream weights). Newton-refined rsqrt."""
    ssum = pool.tile([P, 1], dt.float32, tag="ssum", name="ssum")
    nc.vector.reduce_sum(ssum[:], xt_ap, axis=AX.X)
    sq = pool.tile([P, width], dt.float32, tag="sqscratch", name="sq")
    ssq = pool.tile([P, 1], dt.float32, tag="ssq", name="ssq")
    nc.scalar.activation(sq[:], xt_ap, AF.Square, accum_out=ssq[:])
    mu = pool.tile([P, 1], dt.float32, tag="mu", name="mu")
    nc.vector.tensor_scalar_mul(mu[:], ssum[:], 1.0 / width)
    ex2 = pool.tile([P, 1], dt.float32, tag="ex2", name="ex2")
    nc.vector.tensor_scalar_mul(ex2[:], ssq[:], 1.0 / width)
    mu2 = pool.tile([P, 1], dt.float32, tag="mu2", name="mu2")
    nc.vector.tensor_mul(mu2[:], mu[:], mu[:])
    ve = pool.tile([P, 1], dt.float32, tag="ve", name="ve")
    nc.vector.tensor_sub(ve[:], ex2[:], mu2[:])
    nc.vector.tensor_scalar_add(ve[:], ve[:], EPS)
    s0 = pool.tile([P, 1], dt.float32, tag="s0", name="s0")
    nc.scalar.activation(s0[:], ve[:], AF.Sqrt)
    r0 = pool.tile([P, 1], dt.float32, tag="r0", name="r0")
    nc.vector.reciprocal(r0[:], s0[:])
    vr = pool.tile([P, 1], dt.float32, tag="vr", name="vr")
    nc.vector.tensor_mul(vr[:], ve[:], r0[:])
    s1h = pool.tile([P, 1], dt.float32, tag="s1h", name="s1h")
    nc.vector.tensor_add(s1h[:], s0[:], vr[:])  # = 2*sqrt(ve) after Newton
    rsig = pool.tile([P, 1], dt.float32, tag="rsig", name="rsig")
    nc.vector.reciprocal(rsig[:], s1h[:])
    nc.vector.tensor_scalar_mul(rsig[:], rsig[:], 2.0)
    nmr = pool.tile([P, 1], dt.float32, tag="nmr", name="nmr")
    nc.vector.tensor_scalar(nmr[:], mu[:], rsig[:], -1.0, OP.mult, OP.mult)
    return rsig, nmr


def _build_program(flags):
    has_bq, has_bk, has_bv, has_bg, has_bn, has_be, has_vb, has_w2b = flags
    f32 = dt.float32
    bf16 = dt.bfloat16
    fr = dt.float32r if USE_F32R else dt.float32

    nc = bacc.Bacc("TRN2", target_bir_lowering=False, debug=False,
                   num_devices=8)

    # ---- I/O ----
    # Weights arrive pre-tiled (host rearrange) so every DMA below reads one
    # fully-contiguous DRAM block — avoids the 512B-per-packet strided reads
    # that made the first version DMA-bound.
    xb = nc.dram_tensor("xb", [NB, M], f32, kind="ExternalInput")
    xpb = nc.dram_tensor("xpb", [TL, M], f32, kind="ExternalInput")
    noise_d = nc.dram_tensor("noise", [TL, E], f32, kind="ExternalInput")
    wq_d = nc.dram_tensor("wq", [MT, MT, P, P], f32, kind="ExternalInput")
    wk_d = nc.dram_tensor("wk", [KV // P, MT, P, P], f32, kind="ExternalInput")
    wv_d = nc.dram_tensor("wv", [M, KV], f32, kind="ExternalInput")
    wo_d = nc.dram_tensor("wo", [2, MT, P, 512], f32, kind="ExternalInput")
    wg_d = nc.dram_tensor("wg", [M, E], f32, kind="ExternalInput")
    wn_d = nc.dram_tensor("wn", [M, E], f32, kind="ExternalInput")
    we_d = nc.dram_tensor("we", [E, 2, MT, P, DH // 2], bf16,
                          kind="ExternalInput")
    vw_d = nc.dram_tensor("vw", [2, MT, P, DH // 2], bf16,
                          kind="ExternalInput")
    w2_d = nc.dram_tensor("w2", [DH, M], bf16, kind="ExternalInput")
    bqT_d = bkT_d = bvr_d = bgr_d = bnr_d = ber_d = vbr_d = w2br_d = None
    if has_bq:
        bqT_d = nc.dram_tensor("bqT", [P, MT], f32, kind="ExternalInput")
    if has_bk:
        bkT_d = nc.dram_tensor("bkT", [P, KV // P], f32, kind="ExternalInput")
    if has_bv:
        bvr_d = nc.dram_tensor("bvr", [P, KV], f32, kind="ExternalInput")
    if has_bg:
        bgr_d = nc.dram_tensor("bgr", [P, E], f32, kind="ExternalInput")
    if has_bn:
        bnr_d = nc.dram_tensor("bnr", [P, E], f32, kind="ExternalInput")
    if has_be:
        ber_d = nc.dram_tensor("ber", [E, DH], f32, kind="ExternalInput")
    if has_vb:
        vbr_d = nc.dram_tensor("vbr", [P, DH], f32, kind="ExternalInput")
    if has_w2b:
        w2br_d = nc.dram_tensor("w2br", [P, M], f32, kind="ExternalInput")

    out_d = nc.dram_tensor("out", [TL, M], f32, kind="ExternalOutput")
    u_d = nc.dram_tensor("u", [TL, E], f32, kind="ExternalOutput")
    gates_d = nc.dram_tensor("gates", [TL, E], f32, kind="ExternalOutput")

    from contextlib import ExitStack

    with tile.TileContext(nc) as tc, ExitStack() as top:
        const = top.enter_context(tc.tile_pool(name="const", bufs=1))
        id32 = const.tile([P, P], f32, name="id32")
        make_identity(nc, id32[:])
        idb = const.tile([P, P], bf16, name="idb")
        make_identity(nc, idb[:])

        # persistent activation buffers
        persist = top.enter_context(tc.tile_pool(name="persist", bufs=1))
        x2_all = persist.tile([P, TT * M], f32, name="x2_all")

        # ============ Phase 1: LN1 over the full batch ============
        with ExitStack() as ph:
            xpbp = ph.enter_context(tc.tile_pool(name="xpbp", bufs=1))
            xpb_all = xpbp.tile([P, TT * M], f32, name="xpb_all")
            for t in range(TT):
                nc.sync.dma_start(xpb_all[:, t * M:(t + 1) * M],
                                  xpb[t * P:(t + 1) * P, :])
            ypool = ph.enter_context(tc.tile_pool(name="ypool", bufs=1))
            y_all = ypool.tile([P, NT * M], f32, name="y_all")
            with ExitStack() as inner:
                xin = inner.enter_context(tc.tile_pool(name="xin", bufs=3))
                lns = inner.enter_context(tc.tile_pool(name="lns", bufs=3))
                for t in range(NT):
                    xt = xin.tile([P, M], f32, tag="xt", name="xt")
                    nc.sync.dma_start(xt[:], xb[t * P:(t + 1) * P, :])
                    rsig, nmr = _ln_stats(nc, lns, xt[:], M)
                    nc.scalar.activation(y_all[:, t * M:(t + 1) * M], xt[:],
                                         AF.Identity, bias=nmr[:],
                                         scale=rsig[:])

            # ============ Phase 2: y^T ============
            ytp = ph.enter_context(tc.tile_pool(name="ytp", bufs=1))
            yT = ytp.tile([P, NT * M], fr, name="yT")
            with ExitStack() as inner:
                tps = inner.enter_context(
                    tc.tile_pool(name="tps", bufs=4, space="PSUM"))
                for t in range(NT):
                    for mt in range(MT):
                        pst = tps.tile([P, P], f32, tag="pst", name="pst")
                        nc.tensor.transpose(
                            pst[:], y_all[:, t * M + mt * P: t * M + mt * P + P],
                            id32[:])
                        dst = yT[:, mt * NB + t * P: mt * NB + t * P + P]
                        if (t + mt) % 2 == 0:
                            nc.scalar.copy(dst, pst[:])
                        else:
                            nc.vector.tensor_copy(dst, pst[:])

            # ============ Phase 3: q^T, k^T, v ============
            attn = ph.enter_context(tc.tile_pool(name="attn", bufs=1))
            qT = attn.tile([P, MT * TL], fr, name="qT")
            kT = attn.tile([P, (KV // P) * NB], fr, name="kT")
            v_all = attn.tile([P, NT * (NG * (HD + 1))], fr, name="v_all")
            GW = NG * (HD + 1)  # 260 columns per key tile

            with ExitStack() as inner:
                wst = inner.enter_context(tc.tile_pool(name="wst", bufs=4))
                qps = inner.enter_context(
                    tc.tile_pool(name="qps", bufs=2, space="PSUM"))
                bq_sb = None
                if has_bq:
                    bq_sb = attn.tile([P, MT], f32, name="bq_sb")
                    nc.sync.dma_start(bq_sb[:], bqT_d[:, :])
                bk_sb = None
                if has_bk:
                    bk_sb = attn.tile([P, KV // P], f32, name="bk_sb")
                    nc.sync.dma_start(bk_sb[:], bkT_d[:, :])
                bv_sb = None
                if has_bv:
                    bv_sb = attn.tile([P, KV], f32, name="bv_sb")
                    nc.sync.dma_start(bv_sb[:], bvr_d[:, :])

                def wtile(dram_ap, tag):
                    """DMA a weight tile; convert to f32r via DVE if needed."""
                    t0 = wst.tile([P, dram_ap.shape[-1]], f32, tag=tag,
                                  name=tag)
                    nc.sync.dma_start(t0[:], dram_ap)
                    if not USE_F32R:
                        return t0
                    t1 = wst.tile([P, dram_ap.shape[-1]], fr, tag=tag + "r",
                                  name=tag + "r")
                    nc.vector.tensor_copy(t1[:], t0[:])
                    return t1

                # q^T (features on partitions, local 256 tokens on free)
                for ht in range(MT):
                    psq = qps.tile([P, TL], f32, tag="psq", name="psq")
                    for mt in range(MT):
                        wt = wtile(wq_d[ht, mt], "wq")
                        nc.tensor.matmul(psq[:], wt[:],
                                         yT[:, mt * NB: mt * NB + TL],
                                         start=(mt == 0), stop=(mt == MT - 1))
                    if has_bq:
                        nc.scalar.activation(qT[:, ht * TL:(ht + 1) * TL],
                                             psq[:], AF.Identity,
                                             bias=bq_sb[:, ht:ht + 1])
                    else:
                        nc.scalar.copy(qT[:, ht * TL:(ht + 1) * TL], psq[:])

                # k^T (kv features on partitions, all 1024 batch tokens free)
                for kt in range(KV // P):
                    psk = [qps.tile([P, 512], f32, tag=f"psk{ch}",
                                    name=f"psk{ch}") for ch in range(2)]
                    for mt in range(MT):
                        wt = wtile(wk_d[kt, mt], "wk")
                        for ch in range(2):
                            nc.tensor.matmul(
                                psk[ch][:], wt[:],
                                yT[:, mt * NB + ch * 512: mt * NB + ch * 512 + 512],
                                start=(mt == 0), stop=(mt == MT - 1))
                    for ch in range(2):
                        dst = kT[:, kt * NB + ch * 512: kt * NB + ch * 512 + 512]
                        if has_bk:
                            nc.scalar.activation(dst, psk[ch][:], AF.Identity,
                                                 bias=bk_sb[:, kt:kt + 1])
                        else:
                            nc.scalar.copy(dst, psk[ch][:])

                # v natural (batch tokens on partitions) with ones column per group
                wv_sb = attn.tile([P, MT * KV], f32, name="wv_sb")
                for mt in range(MT):
                    nc.sync.dma_start(wv_sb[:, mt * KV:(mt + 1) * KV],
                                      wv_d[mt * P:(mt + 1) * P, :])
                for t in range(NT):
                    psv = qps.tile([P, KV], f32, tag="psv", name="psv")
                    for mt in range(MT):
                        nc.tensor.matmul(
                            psv[:],
                            yT[:, mt * NB + t * P: mt * NB + t * P + P],
                            wv_sb[:, mt * KV:(mt + 1) * KV],
                            start=(mt == 0), stop=(mt == MT - 1))
                    base = t * GW
                    nc.vector.memset(v_all[:, base: base + GW], 1.0)
                    for g in range(NG):
                        dst = v_all[:, base + g * (HD + 1): base + g * (HD + 1) + HD]
                        if has_bv:
                            nc.vector.scalar_tensor_tensor(
                                dst, psv[:, g * HD:(g + 1) * HD], 1.0,
                                bv_sb[:, g * HD:(g + 1) * HD], OP.mult, OP.add)
                        else:
                            nc.vector.tensor_copy(dst, psv[:, g * HD:(g + 1) * HD])

            # ============ Phase 4: attention heads ============
            aT = attn.tile([P, MT * TL], fr, name="aT")
            with ExitStack() as inner:
                ptp = inner.enter_context(tc.tile_pool(name="ptp", bufs=2))
                sps = inner.enter_context(
                    tc.tile_pool(name="sps", bufs=3, space="PSUM"))
                aps = inner.enter_context(
                    tc.tile_pool(name="aps", bufs=2, space="PSUM"))
                hsm = inner.enter_context(tc.tile_pool(name="hsm", bufs=2))
                for h in range(NH):
                    g = h % NG
                    krow = (g % 2) * HD
                    kcol = (g // 2) * NB
                    qrow = (h % 2) * HD
                    qcol = (h // 2) * TL
                    PTt = ptp.tile([P, NT * TL], fr, tag="PT", name="PTt")
                    for k8 in range(NT):
                        pss = sps.tile([P, TL], f32, tag="pss", name="pss")
                        nc.tensor.matmul(
                            pss[:],
                            kT[krow:krow + HD, kcol + k8 * P: kcol + k8 * P + P],
                            qT[qrow:qrow + HD, qcol: qcol + TL],
                            start=True, stop=True)
                        nc.scalar.activation(PTt[:, k8 * TL:(k8 + 1) * TL],
                                             pss[:], AF.Exp, scale=SCALE)
                    psa = aps.tile([HD + 1, TL], f32, tag="psa", name="psa")
                    for k8 in range(NT):
                        nc.tensor.matmul(
                            psa[:],
                            v_all[:, k8 * GW + g * (HD + 1): k8 * GW + (g + 1) * (HD + 1)],
                            PTt[:, k8 * TL:(k8 + 1) * TL],
                            start=(k8 == 0), stop=(k8 == NT - 1))
                    rr = hsm.tile([1, TL], f32, tag="rr", name="rr")
                    nc.vector.reciprocal(rr[:], psa[HD:HD + 1, :])
                    rb = hsm.tile([HD, TL], f32, tag="rb", name="rb")
                    nc.gpsimd.partition_broadcast(rb[:], rr[:])
                    nc.vector.tensor_mul(
                        aT[qrow:qrow + HD, qcol: qcol + TL],
                        psa[0:HD, :], rb[:])

            # ============ Phase 5: Wo + residual ============
            with ExitStack() as inner:
                wst = inner.enter_context(tc.tile_pool(name="wst2", bufs=4))
                wps = inner.enter_context(
                    tc.tile_pool(name="wps", bufs=4, space="PSUM"))
                for mc in range(2):
                    psw = [wps.tile([P, 512], f32, tag=f"psw{t}",
                                    name=f"psw{t}") for t in range(TT)]
                    for kt in range(MT):
                        t0 = wst.tile([P, 512], f32, tag="wo", name="wo_t")
                        nc.sync.dma_start(t0[:], wo_d[mc, kt])
                        if USE_F32R:
                            wt = wst.tile([P, 512], fr, tag="wor", name="wor_t")
                            nc.vector.tensor_copy(wt[:], t0[:])
                        else:
                            wt = t0
                        for t in range(TT):
                            nc.tensor.matmul(
                                psw[t][:],
                                aT[:, kt * TL + t * P: kt * TL + t * P + P],
                                wt[:], start=(kt == 0), stop=(kt == MT - 1))
                    for t in range(TT):
                        nc.vector.tensor_add(
                            x2_all[:, t * M + mc * 512: t * M + mc * 512 + 512],
                            psw[t][:],
                            xpb_all[:, t * M + mc * 512: t * M + mc * 512 + 512])

        # ============ Phase 6: LN2 -> z, z^T ============
        moe = top.enter_context(tc.tile_pool(name="moe", bufs=1))
        zscope = ExitStack()
        zpool = zscope.enter_context(tc.tile_pool(name="zpool", bufs=1))
        zT32 = zpool.tile([P, MT * TL], f32, name="zT32")
        zTb = moe.tile([P, MT * TL], bf16, name="zTb")
        with ExitStack() as inner:
            lns = inner.enter_context(tc.tile_pool(name="lns2", bufs=2))
            zp = inner.enter_context(tc.tile_pool(name="zp", bufs=1))
            z_all = zp.tile([P, TT * M], f32, name="z_all")
            for t in range(TT):
                rsig, nmr = _ln_stats(nc, lns, x2_all[:, t * M:(t + 1) * M], M)
                nc.scalar.activation(z_all[:, t * M:(t + 1) * M],
                                     x2_all[:, t * M:(t + 1) * M],
                                     AF.Identity, bias=nmr[:], scale=rsig[:])
            tps = inner.enter_context(
                tc.tile_pool(name="tps2", bufs=4, space="PSUM"))
            for t in range(TT):
                for mt in range(MT):
                    pst = tps.tile([P, P], f32, tag="pst", name="pst2")
                    nc.tensor.transpose(
                        pst[:], z_all[:, t * M + mt * P: t * M + mt * P + P],
                        id32[:])
                    nc.scalar.copy(
                        zT32[:, mt * TL + t * P: mt * TL + t * P + P], pst[:])
                    nc.vector.tensor_copy(
                        zTb[:, mt * TL + t * P: mt * TL + t * P + P], pst[:])

        # ============ Phase 7: gating ============
        gates_all = moe.tile([P, TT * E], f32, name="gates_all")
        with ExitStack() as inner:
            gsb = inner.enter_context(tc.tile_pool(name="gsb", bufs=2))
            gps = inner.enter_context(
                tc.tile_pool(name="gps", bufs=2, space="PSUM"))
            wg_sb = gsb.tile([P, MT * E], f32, tag="wg", name="wg_sb")
            wn_sb = gsb.tile([P, MT * E], f32, tag="wn", name="wn_sb")
            for mt in range(MT):
                nc.sync.dma_start(wg_sb[:, mt * E:(mt + 1) * E],
                                  wg_d[mt * P:(mt + 1) * P, :])
                nc.sync.dma_start(wn_sb[:, mt * E:(mt + 1) * E],
                                  wn_d[mt * P:(mt + 1) * P, :])
            bg_sb = bn_sb = None
            if has_bg:
                bg_sb = gsb.tile([P, E], f32, tag="bg", name="bg_sb")
                nc.sync.dma_start(bg_sb[:], bgr_d[:, :])
            if has_bn:
                bn_sb = gsb.tile([P, E], f32, tag="bn", name="bn_sb")
                nc.sync.dma_start(bn_sb[:], bnr_d[:, :])

            for t in range(TT):
                psl = gps.tile([P, E], f32, tag="psl", name="psl")
                psn = gps.tile([P, E], f32, tag="psn", name="psn")
                for mt in range(MT):
                    lhsT = zT32[:, mt * TL + t * P: mt * TL + t * P + P]
                    nc.tensor.matmul(psl[:], lhsT, wg_sb[:, mt * E:(mt + 1) * E],
                                     start=(mt == 0), stop=(mt == MT - 1))
                    nc.tensor.matmul(psn[:], lhsT, wn_sb[:, mt * E:(mt + 1) * E],
                                     start=(mt == 0), stop=(mt == MT - 1))

                def gt(tag):
                    return gsb.tile([P, E], f32, tag=tag, name=tag)

                logits = gt("logits")
                if has_bg:
                    nc.vector.tensor_add(logits[:], psl[:], bg_sb[:])
                else:
                    nc.scalar.copy(logits[:], psl[:])
                raw = gt("raw")
                if has_bn:
                    nc.vector.tensor_add(raw[:], psn[:], bn_sb[:])
                else:
                    nc.scalar.copy(raw[:], psn[:])
                # ns = softplus(raw) = relu(raw) + ln(1 + exp(-|raw|))
                t_abs = gt("t_abs")
                nc.scalar.activation(t_abs[:], raw[:], AF.Abs)
                t_exp = gt("t_exp")
                nc.scalar.activation(t_exp[:], t_abs[:], AF.Exp, scale=-1.0)
                t_ln = gt("t_ln")
                nc.scalar.activation(t_ln[:], t_exp[:], AF.Ln, bias=1.0)
                t_rel = gt("t_rel")
                nc.scalar.activation(t_rel[:], raw[:], AF.Relu)
                ns = gt("ns")
                nc.vector.tensor_add(ns[:], t_rel[:], t_ln[:])
                noise_sb = gt("noise_sb")
                nc.sync.dma_start(noise_sb[:], noise_d[t * P:(t + 1) * P, :])
                Hn = gt("Hn")
                nc.vector.tensor_mul(Hn[:], noise_sb[:], ns[:])
                nc.vector.tensor_add(Hn[:], Hn[:], logits[:])

                def col(tag):
                    return gsb.tile([P, 1], f32, tag=tag, name=tag)

                m1 = col("m1")
                nc.vector.reduce_max(m1[:], Hn[:], axis=AX.X)
                mask1 = gt("mask1")
                nc.vector.tensor_scalar(mask1[:], Hn[:], m1[:], None, OP.is_ge)
                Hn2 = gt("Hn2")
                nc.vector.scalar_tensor_tensor(Hn2[:], mask1[:], -BIG, Hn[:],
                                               OP.mult, OP.add)
                m2 = col("m2")
                nc.vector.reduce_max(m2[:], Hn2[:], axis=AX.X)
                mask2 = gt("mask2")
                nc.vector.tensor_scalar(mask2[:], Hn2[:], m2[:], None, OP.is_ge)
                Hn3 = gt("Hn3")
                nc.vector.scalar_tensor_tensor(Hn3[:], mask2[:], -BIG, Hn2[:],
                                               OP.mult, OP.add)
                m3 = col("m3")
                nc.vector.reduce_max(m3[:], Hn3[:], axis=AX.X)
                # gates = mask1*sig(m1-m2) + mask2*(1-sig(m1-m2))
                d = col("d")
                nc.vector.tensor_sub(d[:], m2[:], m1[:])
                ed = col("ed")
                nc.scalar.activation(ed[:], d[:], AF.Exp)
                nc.vector.tensor_scalar_add(ed[:], ed[:], 1.0)
                g1 = col("g1")
                nc.vector.reciprocal(g1[:], ed[:])
                g2 = col("g2")
                nc.scalar.activation(g2[:], g1[:], AF.Copy, bias=1.0,
                                     scale=-1.0)
                t1g = gt("t1g")
                nc.vector.tensor_scalar(t1g[:], mask1[:], g1[:], None, OP.mult)
                nc.vector.scalar_tensor_tensor(
                    gates_all[:, t * E:(t + 1) * E], mask2[:], g2[:], t1g[:],
                    OP.mult, OP.add)
                # psi = m3 + (mask1+mask2)*(m2-m3); u = (logits-psi)/ns
                msum = gt("msum")
                nc.vector.tensor_add(msum[:], mask1[:], mask2[:])
                d23 = col("d23")
                nc.vector.tensor_sub(d23[:], m2[:], m3[:])
                psi = gt("psi")
                nc.vector.tensor_scalar(psi[:], msum[:], d23[:], m3[:],
                                        OP.mult, OP.add)
                lp = gt("lp")
                nc.vector.tensor_sub(lp[:], logits[:], psi[:])
                rns = gt("rns")
                nc.vector.reciprocal(rns[:], ns[:])
                uu = gt("uu")
                nc.vector.tensor_mul(uu[:], lp[:], rns[:])
                nc.sync.dma_start(u_d[t * P:(t + 1) * P, :], uu[:])
                nc.sync.dma_start(gates_d[t * P:(t + 1) * P, :],
                                  gates_all[:, t * E:(t + 1) * E])

        # ============ Phase 8: xV ============
        zscope.close()  # zT32 no longer needed
        svscope = ExitStack()
        svpool = svscope.enter_context(tc.tile_pool(name="svpool", bufs=1))
        s_all = svpool.tile([P, TT * DH], f32, name="s_all")
        xv_all = svpool.tile([P, TT * DH], bf16, name="xv_all")
        nc.gpsimd.memset(s_all[:], 0.0)
        with ExitStack() as inner:
            wst = inner.enter_context(tc.tile_pool(name="wst3", bufs=2))
            xps = inner.enter_context(
                tc.tile_pool(name="xps", bufs=3, space="PSUM"))
            vb_sb = None
            if has_vb:
                vbp = inner.enter_context(tc.tile_pool(name="vbp", bufs=1))
                vb_sb = vbp.tile([P, DH], f32, name="vb_sb")
                nc.sync.dma_start(vb_sb[:], vbr_d[:, :])
            for half in range(2):
                vts = []
                for mt in range(MT):
                    vt = wst.tile([P, DH // 2], bf16, tag=f"vwt{mt}",
                                  name=f"vwt{mt}")
                    nc.sync.dma_start(vt[:], vw_d[half, mt])
                    vts.append(vt)
                for dcl in range(4):
                    dc = half * 4 + dcl
                    for t in range(TT):
                        psx = xps.tile([P, 512], f32, tag="psx", name="psx")
                        for mt in range(MT):
                            nc.tensor.matmul(
                                psx[:],
                                zTb[:, mt * TL + t * P: mt * TL + t * P + P],
                                vts[mt][:, dcl * 512:(dcl + 1) * 512],
                                start=(mt == 0), stop=(mt == MT - 1))
                        dst = xv_all[:, t * DH + dc * 512: t * DH + dc * 512 + 512]
                        if has_vb:
                            nc.vector.tensor_add(
                                dst, psx[:], vb_sb[:, dc * 512:(dc + 1) * 512])
                        else:
                            nc.scalar.copy(dst, psx[:])

        # ============ Phase 9: experts (dense, bf16) ============
        with ExitStack() as inner:
            wst = inner.enter_context(tc.tile_pool(name="wst4", bufs=2))
            eps_ = inner.enter_context(
                tc.tile_pool(name="eps", bufs=3, space="PSUM"))
            spool = inner.enter_context(tc.tile_pool(name="spool", bufs=3))
            be_sb = None
            if has_be:
                bep = inner.enter_context(tc.tile_pool(name="bep", bufs=2))
            for e in range(E):
                if has_be:
                    ber_row = bep.tile([1, DH], f32, tag="ber_row",
                                       name="ber_row")
                    nc.sync.dma_start(ber_row[:], ber_d[e:e + 1, :])
                    be_sb = bep.tile([P, DH], f32, tag="be_sb", name="be_sb")
                    nc.gpsimd.partition_broadcast(be_sb[:], ber_row[:])
                for half in range(2):
                    wts = []
                    for mt in range(MT):
                        wt = wst.tile([P, DH // 2], bf16, tag=f"wet{mt}",
                                      name=f"wet{mt}")
                        nc.sync.dma_start(wt[:], we_d[e, half, mt])
                        wts.append(wt)
                    for dcl in range(4):
                        dc = half * 4 + dcl
                        for t in range(TT):
                            pse = eps_.tile([P, 512], f32, tag="pse",
                                            name="pse")
                            for mt in range(MT):
                                nc.tensor.matmul(
                                    pse[:],
                                    zTb[:, mt * TL + t * P: mt * TL + t * P + P],
                                    wts[mt][:, dcl * 512:(dcl + 1) * 512],
                                    start=(mt == 0), stop=(mt == MT - 1))
                            if has_be:
                                nc.vector.tensor_add(
                                    pse[:], pse[:],
                                    be_sb[:, dc * 512:(dc + 1) * 512])
                            sil = spool.tile([P, 512], bf16, tag="sil",
                                             name="sil")
                            nc.scalar.activation(sil[:], pse[:], AF.Silu)
                            sl = s_all[:, t * DH + dc * 512:
                                       t * DH + dc * 512 + 512]
                            nc.vector.scalar_tensor_tensor(
                                sl, sil[:],
                                gates_all[:, t * E + e: t * E + e + 1],
                                sl, OP.mult, OP.add)

        # ============ Phase 10: gated, gated^T ============
        gT = moe.tile([P, DT * TL], bf16, name="gT")
        with ExitStack() as inner:
            gp = inner.enter_context(tc.tile_pool(name="gp", bufs=2))
            tps = inner.enter_context(
                tc.tile_pool(name="tps3", bufs=4, space="PSUM"))
            for t in range(TT):
                gtmp = gp.tile([P, DH], bf16, tag="gtmp", name="gtmp")
                nc.vector.tensor_mul(gtmp[:], s_all[:, t * DH:(t + 1) * DH],
                                     xv_all[:, t * DH:(t + 1) * DH])
                for dtt in range(DT):
                    pst = tps.tile([P, P], bf16, tag="pst", name="pst3")
                    nc.tensor.transpose(pst[:], gtmp[:, dtt * P:(dtt + 1) * P],
                                        idb[:])
                    dst = gT[:, dtt * TL + t * P: dtt * TL + t * P + P]
                    if dtt % 2 == 0:
                        nc.scalar.copy(dst, pst[:])
                    else:
                        nc.vector.tensor_copy(dst, pst[:])
        svscope.close()  # s_all / xv_all no longer needed

        # ============ Phase 11: W2 + residual out ============
        with ExitStack() as inner:
            wst = inner.enter_context(tc.tile_pool(name="wst5", bufs=4))
            fps = inner.enter_context(
                tc.tile_pool(name="fps", bufs=1, space="PSUM"))
            opool = inner.enter_context(tc.tile_pool(name="opool", bufs=3))
            w2b_sb = None
            if has_w2b:
                wbp = inner.enter_context(tc.tile_pool(name="wbp", bufs=1))
                w2b_sb = wbp.tile([P, M], f32, name="w2b_sb")
                nc.sync.dma_start(w2b_sb[:], w2br_d[:, :])
            psf = {(mc, t): fps.tile([P, 512], f32, tag=f"psf{mc}_{t}",
                                     name=f"psf{mc}_{t}")
                   for mc in range(2) for t in range(TT)}
            for dtt in range(DT):
                w2t = wst.tile([P, M], bf16, tag="w2t", name="w2t")
                nc.sync.dma_start(w2t[:], w2_d[dtt * P:(dtt + 1) * P, :])
                for mc in range(2):
                    for t in range(TT):
                        nc.tensor.matmul(
                            psf[mc, t][:],
                            gT[:, dtt * TL + t * P: dtt * TL + t * P + P],
                            w2t[:, mc * 512:(mc + 1) * 512],
                            start=(dtt == 0), stop=(dtt == DT - 1))
            for mc in range(2):
                for t in range(TT):
                    ot = opool.tile([P, 512], f32, tag="ot", name="ot")
                    nc.vector.tensor_add(
                        ot[:], psf[mc, t][:],
                        x2_all[:, t * M + mc * 512: t * M + mc * 512 + 512])
                    if has_w2b:
                        nc.vector.tensor_add(
                            ot[:], ot[:], w2b_sb[:, mc * 512:(mc + 1) * 512])
                    nc.sync.dma_start(
                        out_d[t * P:(t + 1) * P, mc * 512:(mc + 1) * 512],
                        ot[:])

    nc.compile()
    return nc


def _get_program(flags):
    if flags not in _programs:
        _programs[flags] = _build_program(flags)
    return _programs[flags]


def kernel(**inputs):
    f32 = np.float32
    bf = ml_dtypes.bfloat16
    x = np.ascontiguousarray(np.asarray(inputs["x"], dtype=f32))
    noise = np.asarray(inputs["noise"], dtype=f32)
    ln1_g = np.asarray(inputs["ln1_g"], f32)
    ln1_b = np.asarray(inputs["ln1_b"], f32)
    ln2_g = np.asarray(inputs["ln2_g"], f32)
    ln2_b = np.asarray(inputs["ln2_b"], f32)
    Wq = np.asarray(inputs["Wq"], f32)
    bq = np.asarray(inputs["bq"], f32)
    Wk = np.asarray(inputs["Wk"], f32)
    bk = np.asarray(inputs["bk"], f32)
    Wv = np.asarray(inputs["Wv"], f32)
    bv = np.asarray(inputs["bv"], f32)
    Wo = np.asarray(inputs["Wo"], f32)
    bo = np.asarray(inputs["bo"], f32)
    Wg = np.asarray(inputs["Wg"], f32)
    bg = np.asarray(inputs["bg"], f32)
    Wn = np.asarray(inputs["Wn"], f32)
    bn = np.asarray(inputs["bn"], f32)
    We = np.asarray(inputs["We"], f32)
    be = np.asarray(inputs["be"], f32)
    Vw = np.asarray(inputs["Vw"], f32)
    Vb = np.asarray(inputs["Vb"], f32)
    W2w = np.asarray(inputs["W2w"], f32)
    W2b = np.asarray(inputs["W2b"], f32)

    # Fold LN gains/biases into the consuming weights (exact when g=1, b=0).
    triv1 = np.all(ln1_g == 1.0) and np.all(ln1_b == 0.0)
    if not triv1:
        bq = bq + ln1_b @ Wq
        bk = bk + ln1_b @ Wk
        bv = bv + ln1_b @ Wv
        Wq = ln1_g[:, None] * Wq
        Wk = ln1_g[:, None] * Wk
        Wv = ln1_g[:, None] * Wv
    triv2 = np.all(ln2_g == 1.0) and np.all(ln2_b == 0.0)
    if not triv2:
        bg = bg + ln2_b @ Wg
        bn = bn + ln2_b @ Wn
        Vb = Vb + ln2_b @ Vw
        be = be + np.einsum("m,emd->ed", ln2_b, We).astype(f32)
        Wg = ln2_g[:, None] * Wg
        Wn = ln2_g[:, None] * Wn
        Vw = ln2_g[:, None] * Vw
        We = ln2_g[None, :, None] * We
    flags = (bool(np.any(bq)), bool(np.any(bk)), bool(np.any(bv)),
             bool(np.any(bg)), bool(np.any(bn)), bool(np.any(be)),
             bool(np.any(Vb)), bool(np.any(W2b)))
    nc = _get_program(flags)

    # Pre-tile weights so each device DMA reads one contiguous block.
    we_b = np.ascontiguousarray(
        We.astype(bf).reshape(E, MT, P, 2, DH // 2).transpose(0, 3, 1, 2, 4))
    vw_b = np.ascontiguousarray(
        Vw.astype(bf).reshape(MT, P, 2, DH // 2).transpose(2, 0, 1, 3))
    w2_b = np.ascontiguousarray(W2w.astype(bf))
    wq_c = np.ascontiguousarray(
        Wq.reshape(MT, P, MT, P).transpose(2, 0, 1, 3))
    wk_c = np.ascontiguousarray(
        Wk.reshape(MT, P, KV // P, P).transpose(2, 0, 1, 3))
    wv_c = np.ascontiguousarray(Wv)
    wo_c = np.ascontiguousarray(
        Wo.reshape(MT, P, 2, 512).transpose(2, 0, 1, 3))
    wg_c = np.ascontiguousarray(Wg)
    wn_c = np.ascontiguousarray(Wn)

    shared = {"wq": wq_c, "wk": wk_c, "wv": wv_c, "wo": wo_c,
              "wg": wg_c, "wn": wn_c, "we": we_b, "vw": vw_b, "w2": w2_b}
    if flags[0]:
        shared["bqT"] = np.ascontiguousarray(bq.reshape(MT, P).T.astype(f32))
    if flags[1]:
        shared["bkT"] = np.ascontiguousarray(bk.reshape(KV // P, P).T.astype(f32))
    if flags[2]:
        shared["bvr"] = np.ascontiguousarray(
            np.broadcast_to(bv, (P, KV)).astype(f32))
    if flags[3]:
        shared["bgr"] = np.ascontiguousarray(
            np.broadcast_to(bg, (P, E)).astype(f32))
    if flags[4]:
        shared["bnr"] = np.ascontiguousarray(
            np.broadcast_to(bn, (P, E)).astype(f32))
    if flags[5]:
        shared["ber"] = np.ascontiguousarray(be.astype(f32))
    if flags[6]:
        shared["vbr"] = np.ascontiguousarray(
            np.broadcast_to(Vb, (P, DH)).astype(f32))
    if flags[7]:
        shared["w2br"] = np.ascontiguousarray(
            np.broadcast_to(W2b, (P, M)).astype(f32))

    in_maps = []
    for c in range(8):
        b = c // 4
        r = c % 4
        xb_c = np.ascontiguousarray(np.roll(x[b], -TL * r, axis=0))
        m = dict(shared)
        m["xb"] = xb_c
        m["xpb"] = np.ascontiguousarray(xb_c[:TL] + bo)
        m["noise"] = np.ascontiguousarray(noise[c * TL:(c + 1) * TL])
        in_maps.append(m)

    trace = os.environ.get("KERNEL_TRACE") == "1"
    res = run_bass_kernel_spmd(nc, in_maps, core_ids=list(range(8)),
                               trace=trace)
    kernel.last_results = res

    out = np.empty((2048, M), dtype=f32)
    u_full = np.empty((2048, E), dtype=f32)
    gates_full = np.empty((2048, E), dtype=f32)
    for c in range(8):
        out[c * TL:(c + 1) * TL] = res.results[c]["out"]
        u_full[c * TL:(c + 1) * TL] = res.results[c]["u"]
        gates_full[c * TL:(c + 1) * TL] = res.results[c]["gates"]

    from scipy.special import ndtr

    def _cv(v):
        v = v.astype(np.float64)
        return v.std() / (v.mean() + 1e-6)

    Pm = ndtr(u_full.astype(np.float64))
    loss = f32(0.01 * _cv(gates_full.sum(0)) + 0.01 * _cv(Pm.sum(0)))
    return (out.reshape(2, 1024, M), loss)
